# revision 1
# baseline (speedup 1.0000x reference)
"""Graphormer-expert GNN kernel for 8 Trainium2 NeuronCores.

Strategy (matches the sharding hint): nodes are partitioned 8 x 6250 (graph
parallel); each core owns the edges whose *target* falls in its shard, so the
scatter-softmax is core-local.  Per layer each core computes LN + the four
projections for its own nodes, the k|v rows are exchanged with an AllGather
collective, and per-edge k/v/q rows are fetched with SWDGE dma_gather
(int16 indices -> the source table is split in two 25088-row buckets).
Per-edge softmax runs without max-subtraction (|alpha| << 1 for this model),
and the segment sums (softmax denominator + message aggregation) are done on
the TensorEngine with host-precomputed 0/1 segment matrices, accumulating all
of one target-block's edge tiles in PSUM.  The softmax division is commuted
past the segment sum and applied per node.
"""

import sys

sys.path.insert(0, "/opt/trn_rl_repo")

import numpy as np

N, IN_DIM, D, H, L, E, MAX_DEG = 50000, 128, 128, 16, 3, 800000, 512
C = D // H
P = 128
NCORES = 8
NSH = N // NCORES            # 6250 nodes per core
NBLK = (NSH + P - 1) // P    # 49 target blocks per core
NPAD = NBLK * P              # 6272 padded rows per core
NB_ROWS = NCORES * NPAD // 2  # 25088 rows per src bucket (fits int16)


def _bf16(a):
    import ml_dtypes

    return np.asarray(a, dtype=ml_dtypes.bfloat16)


def _wrap_idx16(idx, pad_to=None):
    """int16 idx array -> [128, n/16] wrapped (j -> [j%16, j//16]) and
    replicated across the 8 gpsimd cores' 16-partition groups."""
    n = len(idx) if pad_to is None else pad_to
    assert n % 16 == 0
    a = np.zeros(n, dtype=np.int16)
    a[: len(idx)] = idx.astype(np.int16)
    w = a.reshape(n // 16, 16).T  # [16, n/16]
    return np.tile(w, (8, 1))  # [128, n/16]


def _preprocess(x, edge_index):
    """Host-side integer/index preprocessing + per-core shard arrays."""
    src = np.asarray(edge_index[0], dtype=np.int64)
    tgt = np.asarray(edge_index[1], dtype=np.int64)

    # degrees (int) for the centrality-embedding gather
    idg = np.clip(np.bincount(tgt, minlength=N), 0, MAX_DEG)
    odg = np.clip(np.bincount(src, minlength=N), 0, MAX_DEG)

    # global row in the AllGather'ed kv table of node g
    kv_row = (src // NSH) * NPAD + (src % NSH)
    bucket = (kv_row >= NB_ROWS).astype(np.int64)
    src_loc = kv_row - bucket * NB_ROWS  # 0..25087, int16-safe

    cores = []
    # first pass: find the max (block,bucket) run length across all cores
    run_max = 0
    per_core = []
    for c in range(NCORES):
        m = (tgt // NSH) == c
        cs, ct, cb, csl = src[m], tgt[m] - c * NSH, bucket[m], src_loc[m]
        blk = ct // P
        cnt = np.bincount(blk * 2 + cb, minlength=NBLK * 2)
        run_max = max(run_max, cnt.max())
        per_core.append((cs, ct, cb, csl, blk))
    trun = int((run_max + P - 1) // P)  # tiles per (block,bucket) run
    nrun = trun * P
    nb = NBLK * nrun                    # edges per bucket array (padded)

    for c in range(NCORES):
        cs, ct, cb, csl, blk = per_core[c]
        order = np.lexsort((ct, cb, blk))
        cs, ct, cb, csl, blk = (a[order] for a in (cs, ct, cb, csl, blk))

        kv_idx = np.zeros((2, nb), dtype=np.int64)
        S = np.zeros((2, P, nb), dtype=np.float32)  # [bucket, edge%128, ...]
        ST = np.zeros((2, P, nb), dtype=np.float32)  # [bucket, tgt%128, edge pos]
        for b in range(2):
            for k in range(NBLK):
                sel = (cb == b) & (blk == k)
                n_e = int(sel.sum())
                off = k * nrun
                kv_idx[b, off : off + n_e] = csl[sel]
                # padded tail: idx 0 (valid garbage row), S column zero
                tl = ct[sel] - k * P  # 0..127 col within the block
                ee = np.arange(n_e)
                S[b, (off + ee) % P, (off + ee) // P * P + tl] = 1.0
                # note: S stored partition-major: S[b, p, t*128 + col]
                ST[b, tl, off + ee] = 1.0

        cores.append(
            dict(
                kv_idx0=_wrap_idx16(kv_idx[0]),
                kv_idx1=_wrap_idx16(kv_idx[1]),
                st0=_bf16(ST[0]),
                st1=_bf16(ST[1]),
                sm0=_bf16(S[0]),
                sm1=_bf16(S[1]),
                idg=_wrap_idx16(np.pad(idg[c * NSH : (c + 1) * NSH], (0, NPAD - NSH))),
                odg=_wrap_idx16(np.pad(odg[c * NSH : (c + 1) * NSH], (0, NPAD - NSH))),
                x=np.pad(
                    np.asarray(x[c * NSH : (c + 1) * NSH], dtype=np.float32),
                    ((0, NPAD - NSH), (0, 0)),
                ),
            )
        )
    return cores, trun, nb


PROBE_NO_COLLECTIVE = False
import os as _os

GBATCH = int(_os.environ.get("KB_GBATCH", "3"))   # kv/q gather tiles per call
EBATCH = int(_os.environ.get("KB_EBATCH", "3"))   # emb gather blocks per call


def _build(trun, nb):
    from concourse import bass, mybir
    import concourse.tile as tile
    from concourse.bacc import Bacc
    from concourse.masks import make_identity

    dt = mybir.dt
    AX = mybir.AxisListType
    OP = mybir.AluOpType
    AF = mybir.ActivationFunctionType

    nc = Bacc(None, target_bir_lowering=False, debug=False, num_devices=NCORES,
              num_swdge_queues=4)
    qctr = [0]

    def _nextq():
        qctr[0] = (qctr[0] + 1) % 4
        return qctr[0]

    # ---- parameters (per core) -------------------------------------------
    xin = nc.declare_dram_parameter("x", [NPAD, D], dt.float32, isOutput=False)
    emb_i = nc.declare_dram_parameter("emb_in", [MAX_DEG + 1, D], dt.float32, isOutput=False)
    emb_o = nc.declare_dram_parameter("emb_out", [MAX_DEG + 1, D], dt.float32, isOutput=False)
    idg_p = nc.declare_dram_parameter("idg", [P, NPAD // 16], dt.int16, isOutput=False)
    odg_p = nc.declare_dram_parameter("odg", [P, NPAD // 16], dt.int16, isOutput=False)
    win_p = nc.declare_dram_parameter("win", [D, D], dt.bfloat16, isOutput=False)
    bin_p = nc.declare_dram_parameter("bin", [P, D], dt.float32, isOutput=False)
    wcat_p = nc.declare_dram_parameter("wcat", [D, L * 4 * D], dt.bfloat16, isOutput=False)
    bcat_p = nc.declare_dram_parameter("bcat", [P, L * 4 * D], dt.float32, isOutput=False)
    lnp_p = nc.declare_dram_parameter("lnp", [P, L * 2 * D], dt.float32, isOutput=False)
    fnp_p = nc.declare_dram_parameter("fnp", [P, 2 * D], dt.float32, isOutput=False)
    wb_p = nc.declare_dram_parameter("wbeta", [P, L * 2 * D], dt.float32, isOutput=False)
    kvi_p = [
        nc.declare_dram_parameter(f"kv_idx{b}", [P, nb // 16], dt.int16, isOutput=False)
        for b in range(2)
    ]
    st_p = [
        nc.declare_dram_parameter(f"st{b}", [P, nb], dt.bfloat16, isOutput=False)
        for b in range(2)
    ]
    sm_p = [
        nc.declare_dram_parameter(f"sm{b}", [P, nb], dt.bfloat16, isOutput=False)
        for b in range(2)
    ]
    out_p = nc.declare_dram_parameter("out", [NSH, D], dt.float32, isOutput=True)

    # ---- DRAM scratch -----------------------------------------------------
    kvb = nc.dram_tensor("kv_bounce", [NPAD, 2 * D], dt.float8e4)
    kvf = nc.dram_tensor("kv_full", [NCORES * NPAD, 2 * D], dt.float8e4, addr_space="Shared")

    with tile.TileContext(nc) as tc:
        with (
            tc.tile_pool(name="persist", bufs=1) as pp,
            tc.tile_pool(name="wtiles", bufs=1) as wp,
            tc.tile_pool(name="work", bufs=1) as kp,
            tc.tile_pool(name="small", bufs=3) as sp,
            tc.tile_pool(name="edge", bufs=4) as ep,
            tc.tile_pool(name="psA", bufs=2, space="PSUM") as psA,
            tc.tile_pool(name="psB", bufs=2, space="PSUM") as psB,
            tc.tile_pool(name="psC", bufs=1, space="PSUM") as psC,
            tc.tile_pool(name="psQ", bufs=1, space="PSUM") as psQ,
        ):
            # persistent state
            h = pp.tile([P, NBLK, D], dt.float32, tag="h")
            xr = pp.tile([P, NBLK, D], dt.float32, tag="xr")
            qsb = pp.tile([P, NBLK, D], dt.bfloat16, tag="qsb")

            ident = wp.tile([P, P], dt.bfloat16, tag="ident")
            make_identity(nc, ident[:])
            win = wp.tile([D, D], dt.bfloat16, tag="win")
            nc.sync.dma_start(win[:], win_p.ap())
            bin_t = wp.tile([P, D], dt.float32, tag="bin")
            nc.sync.dma_start(bin_t[:], bin_p.ap())
            wcat = wp.tile([D, L, 4 * D], dt.bfloat16, tag="wcat")
            nc.sync.dma_start(wcat[:], wcat_p.ap())
            bcat = wp.tile([P, L, 4 * D], dt.float32, tag="bcat")
            nc.sync.dma_start(bcat[:], bcat_p.ap())
            lnp = wp.tile([P, L, 2 * D], dt.float32, tag="lnp")
            nc.sync.dma_start(lnp[:], lnp_p.ap())
            fnp = wp.tile([P, 2 * D], dt.float32, tag="fnp")
            nc.sync.dma_start(fnp[:], fnp_p.ap())
            wb = wp.tile([P, L, 2 * D], dt.float32, tag="wb")
            nc.sync.dma_start(wb[:], wb_p.ap())

            def _lnproj_block(t, layer):
                """LN h[:,t] (lnp[layer]) -> proj (wcat[layer]) -> kvb/qsb/xr."""
                stats = sp.tile([P, 6], dt.float32, tag="bst")
                nc.vector.bn_stats(stats[:], h[:, t, :])
                mv = sp.tile([P, 2], dt.float32, tag="mv")
                nc.vector.bn_aggr(mv[:], stats[:])
                ve = sp.tile([P, 1], dt.float32, tag="ve")
                nc.vector.tensor_scalar_add(ve[:], mv[:, 1:2], 1e-5)
                sd = sp.tile([P, 1], dt.float32, tag="sd")
                nc.scalar.sqrt(sd[:], ve[:])
                rs = sp.tile([P, 1], dt.float32, tag="rs")
                nc.vector.reciprocal(rs[:], sd[:])
                hf = sp.tile([P, D], dt.float32, tag="hf")
                nc.vector.tensor_tensor(
                    out=hf[:], in0=h[:, t, :],
                    in1=mv[:, 0:1].to_broadcast([P, D]), op=OP.subtract,
                )
                hb = sp.tile([P, D], dt.bfloat16, tag="hb")
                nc.vector.scalar_tensor_tensor(
                    out=hb[:], in0=hf[:], scalar=rs[:], in1=lnp[:, layer, 0:D],
                    op0=OP.mult, op1=OP.mult,
                )
                nc.vector.tensor_tensor(
                    out=hb[:], in0=hb[:], in1=lnp[:, layer, D : 2 * D], op=OP.add
                )
                pT = psA.tile([P, P], dt.bfloat16, tag="pT")
                nc.tensor.transpose(out=pT[:], in_=hb[:], identity=ident[:])
                hnTt = sp.tile([P, D], dt.bfloat16, tag="hnTt")
                nc.scalar.copy(hnTt[:], pT[:])
                ps = psB.tile([P, 4 * D], dt.float32, tag="ps")
                nc.tensor.matmul(
                    out=ps[:], lhsT=hnTt[:], rhs=wcat[:, layer, :],
                    start=True, stop=True,
                )
                kvq = sp.tile([P, 2 * D], dt.float8e4, tag="kvq")
                nc.vector.scalar_tensor_tensor(
                    out=kvq[:], in0=ps[:, 0 : 2 * D], scalar=1.0,
                    in1=bcat[:, layer, 0 : 2 * D], op0=OP.mult, op1=OP.add,
                )
                nc.vector.scalar_tensor_tensor(
                    out=qsb[:, t, :], in0=ps[:, 2 * D : 3 * D], scalar=1.0,
                    in1=bcat[:, layer, 2 * D : 3 * D], op0=OP.mult, op1=OP.add,
                )
                nc.vector.scalar_tensor_tensor(
                    out=xr[:, t, :], in0=ps[:, 3 * D : 4 * D], scalar=1.0,
                    in1=bcat[:, layer, 3 * D : 4 * D], op0=OP.mult, op1=OP.add,
                )
                nc.sync.dma_start(kvb.ap()[t * P : (t + 1) * P, :], kvq[:])

            def _final_block(t):
                """Final LN on h[:,t] -> out DMA."""
                stats = sp.tile([P, 6], dt.float32, tag="bst")
                nc.vector.bn_stats(stats[:], h[:, t, :])
                mv = sp.tile([P, 2], dt.float32, tag="mv")
                nc.vector.bn_aggr(mv[:], stats[:])
                ve = sp.tile([P, 1], dt.float32, tag="ve")
                nc.vector.tensor_scalar_add(ve[:], mv[:, 1:2], 1e-5)
                sd = sp.tile([P, 1], dt.float32, tag="sd")
                nc.scalar.sqrt(sd[:], ve[:])
                rs = sp.tile([P, 1], dt.float32, tag="rs")
                nc.vector.reciprocal(rs[:], sd[:])
                ot = sp.tile([P, D], dt.float32, tag="ot")
                nc.vector.tensor_tensor(
                    out=ot[:], in0=h[:, t, :],
                    in1=mv[:, 0:1].to_broadcast([P, D]), op=OP.subtract,
                )
                nc.vector.scalar_tensor_tensor(
                    out=ot[:], in0=ot[:], scalar=rs[:], in1=fnp[:, 0:D],
                    op0=OP.mult, op1=OP.mult,
                )
                nc.vector.tensor_tensor(
                    out=ot[:], in0=ot[:], in1=fnp[:, D : 2 * D], op=OP.add
                )
                lo = t * P
                hi = min((t + 1) * P, NSH)
                if hi > lo:
                    nc.sync.dma_start(out_p.ap()[lo:hi, :], ot[0 : hi - lo, :])

            # ---- phase 0: h = x @ W_in + b_in + emb_in[idg] + emb_out[odg]
            for t in range(NBLK):
                xt = sp.tile([P, D], dt.float32, tag="xt")
                nc.sync.dma_start(xt[:], xin.ap()[t * P : (t + 1) * P, :])
                xb = sp.tile([P, D], dt.bfloat16, tag="xb")
                nc.vector.tensor_copy(xb[:], xt[:])
                pT = psA.tile([P, P], dt.bfloat16, tag="pT")
                nc.tensor.transpose(out=pT[:], in_=xb[:], identity=ident[:])
                xTb = sp.tile([P, D], dt.bfloat16, tag="xTb")
                nc.scalar.copy(xTb[:], pT[:])
                ph = psB.tile([P, D], dt.float32, tag="ph")
                nc.tensor.matmul(out=ph[:], lhsT=xTb[:], rhs=win[:], start=True, stop=True)
                nc.vector.scalar_tensor_tensor(
                    out=h[:, t, :], in0=ph[:], scalar=1.0, in1=bin_t[:],
                    op0=OP.mult, op1=OP.add,
                )
            for tabl, idxp in ((emb_i, idg_p), (emb_o, odg_p)):
                gi = kp.tile([P, NPAD // 16], dt.int16, tag="gidx")
                nc.sync.dma_start(gi[:], idxp.ap())
                eg = kp.tile([P, NBLK, D], dt.float32, tag="scratch")
                for i0 in range(0, NBLK, EBATCH):
                    i1 = min(i0 + EBATCH, NBLK)
                    nidx = (i1 - i0) * P
                    nc.gpsimd.dma_gather(
                        out_ap=eg[:, i0:i1, :], in_ap=tabl.ap(),
                        idxs_ap=gi[:, i0 * 8 : i1 * 8],
                        num_idxs=nidx, num_idxs_reg=nidx, elem_size=D,
                        queue_num=_nextq(),
                    )
                nc.vector.tensor_tensor(out=h[:], in0=h[:], in1=eg[:], op=OP.add)

            # ---- pre-pass: LN + projections for layer 0
            for t in range(NBLK):
                _lnproj_block(t, 0)

            # ---- layers ----------------------------------------------------
            for layer in range(L):
                if PROBE_NO_COLLECTIVE:
                    nc.gpsimd.dma_start(out=kvf.ap()[0:NPAD, :], in_=kvb.ap())
                else:
                    nc.gpsimd.collective_compute(
                        "AllGather",
                        OP.bypass,
                        replica_groups=[list(range(NCORES))],
                        ins=[kvb.ap().opt()],
                        outs=[kvf.ap().opt()],
                    )

                # ---- edge phase: per (tgt block, bucket) run of trun tiles
                for blk in range(NBLK):
                    pm = psC.tile([P, D + H], dt.float32, tag="pm")
                    for b in range(2):
                        e0 = blk * trun * P  # edge offset in bucket arrays
                        i0 = e0 // 16
                        ki = ep.tile([P, trun * P // 16], dt.int16, tag="ki")
                        nc.sync.dma_start(ki[:], kvi_p[b].ap()[:, i0 : i0 + trun * P // 16])
                        sg = ep.tile([P, trun, P], dt.bfloat16, tag="sg")
                        nc.sync.dma_start(sg[:], sm_p[b].ap()[:, e0 : e0 + trun * P])
                        st = ep.tile([P, trun * P], dt.bfloat16, tag="st")
                        nc.scalar.dma_start(st[:], st_p[b].ap()[:, e0 : e0 + trun * P])

                        kvg = ep.tile([P, trun, 2 * D], dt.float8e4, tag="kvg")
                        for i0 in range(0, trun, GBATCH):
                            i1 = min(i0 + GBATCH, trun)
                            nidx = (i1 - i0) * P
                            nc.gpsimd.dma_gather(
                                out_ap=kvg[:, i0:i1, :],
                                in_ap=kvf.ap()[b * NB_ROWS : (b + 1) * NB_ROWS, :],
                                idxs_ap=ki[:, i0 * 8 : i1 * 8],
                                num_idxs=nidx, num_idxs_reg=nidx,
                                elem_size=2 * D,
                                queue_num=_nextq(),
                            )
                        # qT[f, e] = q_blk^T @ ST, chunked to fit PSUM banks
                        qTs = ep.tile([P, trun * P], dt.bfloat16, tag="qTs")
                        for c0 in range(0, trun * P, 512):
                            c1 = min(c0 + 512, trun * P)
                            qTp = psQ.tile([P, 512], dt.float32, tag="qTp")
                            nc.tensor.matmul(
                                out=qTp[:, 0 : c1 - c0], lhsT=qsb[:, blk, :],
                                rhs=st[:, c0:c1], start=True, stop=True,
                            )
                            nc.scalar.copy(qTs[:, c0:c1], qTp[:, 0 : c1 - c0])
                        # xbar transpose back to row-major [e%128, tile, feat]
                        qg = ep.tile([P, trun, P], dt.bfloat16, tag="qg")
                        nc.scalar.dma_start(qg[:], qTs[:], transpose=True)
                        nc.vector.tensor_tensor(
                            out=qg[:], in0=qg[:], in1=kvg[:, :, 0:D], op=OP.mult
                        )
                        al = ep.tile([P, trun, H, 1], dt.bfloat16, tag="al")
                        with nc.allow_low_precision(reason="alpha logits are O(0.1)"):
                            nc.vector.tensor_reduce(
                                out=al[:, :, :, 0:1],
                                in_=qg[:].rearrange("p t (h c) -> p t h c", c=C),
                                axis=AX.X, op=OP.add,
                            )
                        ue = ep.tile([P, trun, D + H], dt.bfloat16, tag="ue")
                        nc.scalar.activation(
                            out=ue[:, :, D : D + H].rearrange("p t (h o) -> p t h o", o=1),
                            in_=al[:],
                            func=AF.Exp,
                        )
                        nc.vector.tensor_tensor(
                            out=ue[:, :, 0:D].rearrange("p t (h c) -> p t h c", c=C),
                            in0=kvg[:, :, D : 2 * D].rearrange("p t (h c) -> p t h c", c=C),
                            in1=ue[:, :, D : D + H]
                            .rearrange("p t (h o) -> p t h o", o=1)
                            .to_broadcast([P, trun, H, C]),
                            op=OP.mult,
                        )
                        for tt in range(trun):
                            nc.tensor.matmul(
                                out=pm[:], lhsT=sg[:, tt, :], rhs=ue[:, tt, :],
                                start=(b == 0 and tt == 0),
                                stop=(b == 1 and tt == trun - 1),
                            )

                    # ---- fused per-block tail: normalize, gate, residual,
                    # then next layer's LN+projection (or final LN) ----------
                    msgb = sp.tile([P, D], dt.float32, tag="msgb")
                    denb = sp.tile([P, H, 1], dt.float32, tag="denb")
                    nc.scalar.copy(denb[:], pm[:, D : D + H].rearrange("p (h o) -> p h o", o=1))
                    rden = sp.tile([P, H, 1], dt.float32, tag="rden")
                    nc.vector.tensor_scalar_add(rden[:], denb[:], 1e-20)
                    nc.vector.reciprocal(rden[:], rden[:])
                    nc.vector.tensor_tensor(
                        out=msgb[:].rearrange("p (h c) -> p h c", c=C),
                        in0=pm[:, 0:D].rearrange("p (h c) -> p h c", c=C),
                        in1=rden[:].to_broadcast([P, H, C]),
                        op=OP.mult,
                    )
                    tmpb = sp.tile([P, D], dt.float32, tag="tmpb")
                    nc.vector.tensor_tensor(
                        out=tmpb[:], in0=msgb[:], in1=wb[:, layer, 0:D], op=OP.mult
                    )
                    bs1 = sp.tile([P, 1], dt.float32, tag="bs1")
                    nc.vector.tensor_reduce(out=bs1[:], in_=tmpb[:], axis=AX.X, op=OP.add)
                    nc.vector.tensor_tensor(
                        out=tmpb[:], in0=xr[:, blk, :], in1=wb[:, layer, D : 2 * D], op=OP.mult
                    )
                    bs2 = sp.tile([P, 1], dt.float32, tag="bs2")
                    nc.vector.tensor_reduce(out=bs2[:], in_=tmpb[:], axis=AX.X, op=OP.add)
                    nc.vector.tensor_tensor(out=bs1[:], in0=bs1[:], in1=bs2[:], op=OP.add)
                    beta = sp.tile([P, 1], dt.float32, tag="beta")
                    nc.scalar.activation(out=beta[:], in_=bs1[:], func=AF.Sigmoid)
                    # h += msg + beta*(xr - msg)
                    nc.vector.tensor_tensor(
                        out=tmpb[:], in0=xr[:, blk, :], in1=msgb[:], op=OP.subtract
                    )
                    nc.vector.scalar_tensor_tensor(
                        out=tmpb[:], in0=tmpb[:], scalar=beta[:], in1=msgb[:],
                        op0=OP.mult, op1=OP.add,
                    )
                    nc.vector.tensor_tensor(
                        out=h[:, blk, :], in0=h[:, blk, :], in1=tmpb[:], op=OP.add
                    )
                    if layer == L - 1:
                        _final_block(blk)
                    else:
                        _lnproj_block(blk, layer + 1)

    nc.finalize()
    return nc

LAST_RES = None


def _make_in_maps(inputs, cores):
    sq = 1.0 / np.sqrt(np.float32(C))
    Wq, Wk, Wv, Wsk = (np.asarray(inputs[k], dtype=np.float32) for k in ("Wq", "Wk", "Wv", "Wskip"))
    bq, bk, bv, bsk = (np.asarray(inputs[k], dtype=np.float32) for k in ("bq", "bk", "bv", "bskip"))
    wcat = np.concatenate([Wk * sq, Wv, Wq, Wsk], axis=2).transpose(1, 0, 2).reshape(D, L * 4 * D)
    bcat = np.concatenate([bk * sq, bv, bq, bsk], axis=1)  # [L,512]
    bcat_rep = np.broadcast_to(bcat[:, None, :], (L, P, 4 * D)).transpose(1, 0, 2).reshape(P, L * 4 * D).copy()
    lns, lnb = np.asarray(inputs["ln_scale"], np.float32), np.asarray(inputs["ln_bias"], np.float32)
    lnp = np.broadcast_to(
        np.concatenate([lns, lnb], axis=1)[:, None, :], (L, P, 2 * D)
    ).transpose(1, 0, 2).reshape(P, L * 2 * D).copy()
    fnp = np.broadcast_to(
        np.concatenate([inputs["fn_scale"], inputs["fn_bias"]])[None, :], (P, 2 * D)
    ).astype(np.float32).copy()
    Wbeta = np.asarray(inputs["Wbeta"], np.float32)  # [L, 3D, 1]
    wa = Wbeta[:, 0:D, 0] + Wbeta[:, 2 * D : 3 * D, 0]      # msg coeff
    wbx = Wbeta[:, D : 2 * D, 0] - Wbeta[:, 2 * D : 3 * D, 0]  # xr coeff
    wbeta_rep = np.broadcast_to(
        np.concatenate([wa, wbx], axis=1)[:, None, :], (L, P, 2 * D)
    ).transpose(1, 0, 2).reshape(P, L * 2 * D).copy()
    bin_rep = np.broadcast_to(
        np.asarray(inputs["b_in"], np.float32)[None, :], (P, D)
    ).copy()

    common = dict(
        emb_in=np.asarray(inputs["in_emb"], np.float32),
        emb_out=np.asarray(inputs["out_emb"], np.float32),
        win=_bf16(inputs["W_in"]),
        bin=bin_rep,
        wcat=_bf16(wcat),
        bcat=bcat_rep,
        lnp=lnp,
        fnp=fnp,
        wbeta=wbeta_rep,
    )
    in_maps = []
    for c in range(NCORES):
        m = dict(common)
        cd = cores[c]
        m.update(
            x=cd["x"], idg=cd["idg"], odg=cd["odg"],
            kv_idx0=cd["kv_idx0"], kv_idx1=cd["kv_idx1"],
            st0=cd["st0"], st1=cd["st1"],
            sm0=cd["sm0"], sm1=cd["sm1"],
        )
        in_maps.append(m)
    return in_maps


def kernel(**inputs):
    import os

    from concourse.bass_utils import run_bass_kernel_spmd

    x = np.asarray(inputs["x"], dtype=np.float32)
    edge_index = np.asarray(inputs["edge_index"])
    cores, trun, nb = _preprocess(x, edge_index)
    in_maps = _make_in_maps(inputs, cores)

    nc = _build(trun, nb)
    kw = {}
    td = os.environ.get("BASS_KERNEL_TMPDIR")
    if td:
        kw["tmpdir"] = td
    res = run_bass_kernel_spmd(nc, in_maps, core_ids=list(range(NCORES)), **kw)
    global LAST_RES
    LAST_RES = res
    outs = [np.asarray(r["out"], dtype=np.float32) for r in res.results]
    return np.concatenate(outs, axis=0)


if __name__ == "__main__":
    import reference

    inp = {k: np.asarray(v) for k, v in reference.setup_inputs().items()}
    exp = np.asarray(reference.reference(**inp))
    act = kernel(**inp)
    err = np.abs(act - exp).max() / (np.abs(exp).max() + 1e-9)
    print("Relative error:", err)



# revision 7
# speedup vs baseline: 1.5302x; 1.5302x over previous
"""Graphormer-expert GNN kernel for 8 Trainium2 NeuronCores.

Strategy (matches the sharding hint): nodes are partitioned 8 x 6250 (graph
parallel); each core owns the edges whose *target* falls in its shard, so the
scatter-softmax is core-local.  Per layer each core computes LN + projections
for its own nodes, k|v rows (bf16) are exchanged with an AllGather, and
per-edge k/v rows are fetched with SWDGE dma_gather (int16 indices; source
table split in two 25088-row buckets; padding indices are -1 so the Q7
descriptor generator trims them).  Per-edge softmax runs without
max-subtraction (|alpha| << 1 for this model); the q-row broadcast and the
segment sums (softmax denominator + message aggregation) run on the
TensorEngine with host-precomputed fp8 one-hot matrices (exact), accumulating
each target-block's edge tiles in PSUM.  The softmax division is commuted
past the segment sum and applied per node.  k-bias drops out exactly (softmax
shift invariance); v-bias is folded in after aggregation.  ACT runs only
copies + Exp (sigmoid via exp, layernorm rsqrt via DVE Newton iteration) so
the activation table is loaded once.
"""

import sys

sys.path.insert(0, "/opt/trn_rl_repo")

import numpy as np

N, IN_DIM, D, H, L, E, MAX_DEG = 50000, 128, 128, 16, 3, 800000, 512
C = D // H
P = 128
NCORES = 8
NSH = N // NCORES            # 6250 nodes per core
NBLK = (NSH + P - 1) // P    # 49 target blocks per core
NPAD = NBLK * P              # 6272 padded rows per core
NB_ROWS = NCORES * NPAD // 2  # 25088 rows per src bucket (fits int16)

RSQRT_MAGIC = 0x5F3759DF


def _bf16(a):
    import ml_dtypes

    return np.asarray(a, dtype=ml_dtypes.bfloat16)


def _fp8(a):
    import ml_dtypes

    return np.asarray(a, dtype=ml_dtypes.float8_e4m3)


def _wrap_idx16(idx, pad_to=None, fill=0):
    """int16 idx array -> [128, n/16] wrapped (j -> [j%16, j//16]) and
    replicated across the 8 gpsimd cores' 16-partition groups."""
    n = len(idx) if pad_to is None else pad_to
    assert n % 16 == 0
    a = np.full(n, fill, dtype=np.int16)
    a[: len(idx)] = idx.astype(np.int16)
    w = a.reshape(n // 16, 16).T  # [16, n/16]
    return np.tile(w, (8, 1))  # [128, n/16]


def _preprocess(x, edge_index):
    """Host-side integer/index preprocessing + per-core shard arrays."""
    src = np.asarray(edge_index[0], dtype=np.int64)
    tgt = np.asarray(edge_index[1], dtype=np.int64)

    # degrees (int) for the centrality-embedding gather
    idg = np.clip(np.bincount(tgt, minlength=N), 0, MAX_DEG)
    odg = np.clip(np.bincount(src, minlength=N), 0, MAX_DEG)

    # global row in the AllGather'ed kv table of node g
    kv_row = (src // NSH) * NPAD + (src % NSH)
    bucket = (kv_row >= NB_ROWS).astype(np.int64)
    src_loc = kv_row - bucket * NB_ROWS  # 0..25087, int16-safe

    # first pass: find the max (block,bucket) run length across all cores
    run_max = 0
    per_core = []
    for c in range(NCORES):
        m = (tgt // NSH) == c
        cs, ct, cb, csl = src[m], tgt[m] - c * NSH, bucket[m], src_loc[m]
        blk = ct // P
        cnt = np.bincount(blk * 2 + cb, minlength=NBLK * 2)
        run_max = max(run_max, cnt.max())
        per_core.append((cs, ct, cb, csl, blk))
    trun = int((run_max + P - 1) // P)  # tiles per (block,bucket) run
    nrun = trun * P
    KIB = trun * 16                     # idx bytes per partition per run
    EDB = KIB + 2 * trun * P            # + sg fp8 + st fp8 bytes

    cores = []
    for c in range(NCORES):
        cs, ct, cb, csl, blk = per_core[c]
        order = np.lexsort((ct, cb, blk))
        cs, ct, cb, csl, blk = (a[order] for a in (cs, ct, cb, csl, blk))

        edata = np.zeros((P, NBLK * 2 * EDB), dtype=np.uint8)
        for k in range(NBLK):
            for b in range(2):
                sel = (cb == b) & (blk == k)
                n_e = int(sel.sum())
                ki = _wrap_idx16(csl[sel], pad_to=nrun, fill=0)  # [128, trun*8]
                tl = ct[sel] - k * P  # 0..127 col within the block
                ee = np.arange(n_e)
                S = np.zeros((P, nrun), dtype=np.float32)   # [e%128, t*128+tl]
                S[ee % P, (ee // P) * P + tl] = 1.0
                ST = np.zeros((P, nrun), dtype=np.float32)  # [tl, e]
                ST[tl, ee] = 1.0
                off = (k * 2 + b) * EDB
                edata[:, off : off + KIB] = ki.view(np.uint8)
                edata[:, off + KIB : off + KIB + nrun] = _fp8(S).view(np.uint8)
                edata[:, off + KIB + nrun : off + EDB] = _fp8(ST).view(np.uint8)

        cores.append(
            dict(
                edata=edata,
                idg=_wrap_idx16(np.pad(idg[c * NSH : (c + 1) * NSH], (0, NPAD - NSH))),
                odg=_wrap_idx16(np.pad(odg[c * NSH : (c + 1) * NSH], (0, NPAD - NSH))),
                x=np.pad(
                    np.asarray(x[c * NSH : (c + 1) * NSH], dtype=np.float32),
                    ((0, NPAD - NSH), (0, 0)),
                ),
            )
        )
    return cores, trun, EDB


import os as _os

PROBE_NO_COLLECTIVE = bool(int(_os.environ.get("KB_NOCOLL", "0")))
ABL_NOEDGE = bool(int(_os.environ.get("KB_NOEDGE", "0")))   # skip edge interior

GBATCH = int(_os.environ.get("KB_GBATCH", "5"))   # kv gather tiles per call
EBATCH = int(_os.environ.get("KB_EBATCH", "3"))   # emb gather blocks per call
EPBUFS = int(_os.environ.get("KB_EPBUFS", "4"))   # edge pool depth


def _build(trun, EDB):
    from concourse import bass, mybir
    import concourse.tile as tile
    from concourse.bacc import Bacc
    from concourse.masks import make_identity

    dt = mybir.dt
    AX = mybir.AxisListType
    OP = mybir.AluOpType
    AF = mybir.ActivationFunctionType

    KIB = trun * 16
    nrun = trun * P

    nc = Bacc(None, target_bir_lowering=False, debug=False, num_devices=NCORES,
              num_swdge_queues=4)
    qctr = [0]

    def _nextq():
        qctr[0] = (qctr[0] + 1) % 4
        return qctr[0]

    # ---- parameters (per core) -------------------------------------------
    xin = nc.declare_dram_parameter("x", [NPAD, D], dt.float32, isOutput=False)
    emb_i = nc.declare_dram_parameter("emb_in", [MAX_DEG + 1, D], dt.float32, isOutput=False)
    emb_o = nc.declare_dram_parameter("emb_out", [MAX_DEG + 1, D], dt.float32, isOutput=False)
    idg_p = nc.declare_dram_parameter("idg", [P, NPAD // 16], dt.int16, isOutput=False)
    odg_p = nc.declare_dram_parameter("odg", [P, NPAD // 16], dt.int16, isOutput=False)
    win_p = nc.declare_dram_parameter("win", [D, D], dt.bfloat16, isOutput=False)
    bin_p = nc.declare_dram_parameter("bin", [P, D], dt.float32, isOutput=False)
    wcat_p = nc.declare_dram_parameter("wcat", [D, L * 4 * D], dt.bfloat16, isOutput=False)
    bcat_p = nc.declare_dram_parameter("bcat", [P, L * 2 * D], dt.float32, isOutput=False)
    bvp_p = nc.declare_dram_parameter("bvp", [P, L * D], dt.float32, isOutput=False)
    lnp_p = nc.declare_dram_parameter("lnp", [P, L * 2 * D], dt.float32, isOutput=False)
    fnp_p = nc.declare_dram_parameter("fnp", [P, 2 * D], dt.float32, isOutput=False)
    wb_p = nc.declare_dram_parameter("wbeta", [P, L * 2 * D], dt.float32, isOutput=False)
    ed_p = nc.declare_dram_parameter("edata", [P, NBLK * 2 * EDB], dt.uint8, isOutput=False)
    out_p = nc.declare_dram_parameter("out", [NSH, D], dt.float32, isOutput=True)

    # ---- DRAM scratch -----------------------------------------------------
    kvb = nc.dram_tensor("kv_bounce", [NPAD, 2 * D], dt.bfloat16)
    kvf = nc.dram_tensor("kv_full", [NCORES * NPAD, 2 * D], dt.bfloat16, addr_space="Shared")

    with tile.TileContext(nc) as tc:
        with (
            tc.tile_pool(name="persist", bufs=1) as pp,
            tc.tile_pool(name="wtiles", bufs=1) as wp,
            tc.tile_pool(name="work", bufs=1) as kp,
            tc.tile_pool(name="small", bufs=3) as sp,
            tc.tile_pool(name="edge", bufs=EPBUFS) as ep,
            tc.tile_pool(name="psA", bufs=1, space="PSUM") as psA,
            tc.tile_pool(name="psB", bufs=1, space="PSUM") as psB,
            tc.tile_pool(name="psC", bufs=2, space="PSUM") as psC,
            tc.tile_pool(name="psQ", bufs=2, space="PSUM") as psQ,
        ):
            # persistent state
            h = pp.tile([P, NBLK, D], dt.float32, tag="h")
            xr = pp.tile([P, NBLK, D], dt.bfloat16, tag="xr")
            qsb = pp.tile([P, NBLK, D], dt.bfloat16, tag="qsb")

            ident = wp.tile([P, P], dt.bfloat16, tag="ident")
            make_identity(nc, ident[:])
            win = wp.tile([D, D], dt.bfloat16, tag="win")
            nc.sync.dma_start(win[:], win_p.ap())
            bin_t = wp.tile([P, D], dt.float32, tag="bin")
            nc.sync.dma_start(bin_t[:], bin_p.ap())
            wcat = wp.tile([D, L, 4 * D], dt.bfloat16, tag="wcat")
            nc.sync.dma_start(wcat[:], wcat_p.ap())
            bcat = wp.tile([P, L, 2 * D], dt.float32, tag="bcat")
            nc.sync.dma_start(bcat[:], bcat_p.ap())
            bvt = wp.tile([P, L, D], dt.float32, tag="bvt")
            nc.sync.dma_start(bvt[:], bvp_p.ap())
            lnp = wp.tile([P, L, 2 * D], dt.float32, tag="lnp")
            nc.sync.dma_start(lnp[:], lnp_p.ap())
            fnp = wp.tile([P, 2 * D], dt.float32, tag="fnp")
            nc.sync.dma_start(fnp[:], fnp_p.ap())
            wb = wp.tile([P, L, 2 * D], dt.float32, tag="wb")
            nc.sync.dma_start(wb[:], wb_p.ap())

            def _rsqrt(rs, ve):
                """rs = 1/sqrt(ve) via bit-hack seed + 2 Newton iterations.
                rs, ve: [P, 1] f32 tiles (DVE only — no ACT table)."""
                iv = sp.tile([P, 1], dt.int32, tag="nw_i")
                nc.vector.tensor_scalar(
                    out=iv[:], in0=ve[:].bitcast(dt.int32), scalar1=1,
                    scalar2=None, op0=OP.logical_shift_right,
                )
                nc.vector.tensor_scalar(
                    out=iv[:], in0=iv[:], scalar1=-1, scalar2=RSQRT_MAGIC,
                    op0=OP.mult, op1=OP.add,
                )
                y = iv[:].bitcast(dt.float32)
                t = sp.tile([P, 1], dt.float32, tag="nw_t")
                for _ in range(2):
                    nc.vector.tensor_tensor(out=t[:], in0=y, in1=y, op=OP.mult)
                    nc.vector.tensor_tensor(out=t[:], in0=t[:], in1=ve[:], op=OP.mult)
                    nc.vector.tensor_scalar(
                        out=t[:], in0=t[:], scalar1=-0.5, scalar2=1.5,
                        op0=OP.mult, op1=OP.add,
                    )
                    nc.vector.tensor_tensor(out=t[:], in0=y, in1=t[:], op=OP.mult)
                    nc.vector.tensor_copy(iv[:].bitcast(dt.float32), t[:])
                nc.vector.tensor_copy(rs[:], y)

            def _ln_to(hb, t, scale_ap, bias_ap):
                """hb[P, D] (bf16) = LN(h[:, t, :]) * scale + bias."""
                stats = sp.tile([P, 6], dt.float32, tag="bst")
                nc.vector.bn_stats(stats[:], h[:, t, :])
                mv = sp.tile([P, 2], dt.float32, tag="mv")
                nc.vector.bn_aggr(mv[:], stats[:])
                ve = sp.tile([P, 1], dt.float32, tag="ve")
                nc.vector.tensor_scalar_add(ve[:], mv[:, 1:2], 1e-5)
                rs = sp.tile([P, 1], dt.float32, tag="rs")
                _rsqrt(rs, ve)
                hf = sp.tile([P, D], dt.float32, tag="hf")
                nc.vector.tensor_tensor(
                    out=hf[:], in0=h[:, t, :],
                    in1=mv[:, 0:1].to_broadcast([P, D]), op=OP.subtract,
                )
                nc.vector.scalar_tensor_tensor(
                    out=hb[:], in0=hf[:], scalar=rs[:], in1=scale_ap,
                    op0=OP.mult, op1=OP.mult,
                )
                nc.vector.tensor_tensor(out=hb[:], in0=hb[:], in1=bias_ap, op=OP.add)

            def _lnproj_block(t, layer):
                """LN h[:,t] (lnp[layer]) -> proj (wcat[layer]) -> kvb/qsb/xr."""
                hb = sp.tile([P, D], dt.bfloat16, tag="hb")
                _ln_to(hb, t, lnp[:, layer, 0:D], lnp[:, layer, D : 2 * D])
                pT = psA.tile([P, P], dt.bfloat16, tag="pT")
                nc.tensor.transpose(out=pT[:], in_=hb[:], identity=ident[:])
                hnTt = sp.tile([P, D], dt.bfloat16, tag="hnTt")
                nc.scalar.copy(hnTt[:], pT[:])
                ps = psB.tile([P, 4 * D], dt.float32, tag="ps")
                nc.tensor.matmul(
                    out=ps[:], lhsT=hnTt[:], rhs=wcat[:, layer, :],
                    start=True, stop=True,
                )
                kvq = sp.tile([P, 2 * D], dt.bfloat16, tag="kvq")
                nc.scalar.copy(kvq[:], ps[:, 0 : 2 * D])
                nc.vector.scalar_tensor_tensor(
                    out=qsb[:, t, :], in0=ps[:, 2 * D : 3 * D], scalar=1.0,
                    in1=bcat[:, layer, 0:D], op0=OP.mult, op1=OP.add,
                )
                nc.vector.scalar_tensor_tensor(
                    out=xr[:, t, :], in0=ps[:, 3 * D : 4 * D], scalar=1.0,
                    in1=bcat[:, layer, D : 2 * D], op0=OP.mult, op1=OP.add,
                )
                nc.sync.dma_start(kvb.ap()[t * P : (t + 1) * P, :], kvq[:])

            def _final_block(t):
                """Final LN on h[:,t] -> out DMA."""
                ot = sp.tile([P, D], dt.float32, tag="ot")
                stats = sp.tile([P, 6], dt.float32, tag="bst")
                nc.vector.bn_stats(stats[:], h[:, t, :])
                mv = sp.tile([P, 2], dt.float32, tag="mv")
                nc.vector.bn_aggr(mv[:], stats[:])
                ve = sp.tile([P, 1], dt.float32, tag="ve")
                nc.vector.tensor_scalar_add(ve[:], mv[:, 1:2], 1e-5)
                rs = sp.tile([P, 1], dt.float32, tag="rs")
                _rsqrt(rs, ve)
                nc.vector.tensor_tensor(
                    out=ot[:], in0=h[:, t, :],
                    in1=mv[:, 0:1].to_broadcast([P, D]), op=OP.subtract,
                )
                nc.vector.scalar_tensor_tensor(
                    out=ot[:], in0=ot[:], scalar=rs[:], in1=fnp[:, 0:D],
                    op0=OP.mult, op1=OP.mult,
                )
                nc.vector.tensor_tensor(
                    out=ot[:], in0=ot[:], in1=fnp[:, D : 2 * D], op=OP.add
                )
                lo = t * P
                hi = min((t + 1) * P, NSH)
                if hi > lo:
                    nc.sync.dma_start(out_p.ap()[lo:hi, :], ot[0 : hi - lo, :])

            # ---- phase 0: h = x @ W_in + b_in + emb_in[idg] + emb_out[odg]
            for t in range(NBLK):
                xt = sp.tile([P, D], dt.float32, tag="xt")
                nc.sync.dma_start(xt[:], xin.ap()[t * P : (t + 1) * P, :])
                xb = sp.tile([P, D], dt.bfloat16, tag="xb")
                nc.vector.tensor_copy(xb[:], xt[:])
                pT = psA.tile([P, P], dt.bfloat16, tag="pT")
                nc.tensor.transpose(out=pT[:], in_=xb[:], identity=ident[:])
                xTb = sp.tile([P, D], dt.bfloat16, tag="xTb")
                nc.scalar.copy(xTb[:], pT[:])
                ph = psB.tile([P, D], dt.float32, tag="ph")
                nc.tensor.matmul(out=ph[:], lhsT=xTb[:], rhs=win[:], start=True, stop=True)
                nc.vector.scalar_tensor_tensor(
                    out=h[:, t, :], in0=ph[:], scalar=1.0, in1=bin_t[:],
                    op0=OP.mult, op1=OP.add,
                )
            for tabl, idxp in ((emb_i, idg_p), (emb_o, odg_p)):
                gi = kp.tile([P, NPAD // 16], dt.int16, tag="gidx")
                nc.sync.dma_start(gi[:], idxp.ap())
                eg = kp.tile([P, NBLK, D], dt.float32, tag="scratch")
                for i0 in range(0, NBLK, EBATCH):
                    i1 = min(i0 + EBATCH, NBLK)
                    nidx = (i1 - i0) * P
                    nc.gpsimd.dma_gather(
                        out_ap=eg[:, i0:i1, :], in_ap=tabl.ap(),
                        idxs_ap=gi[:, i0 * 8 : i1 * 8],
                        num_idxs=nidx, num_idxs_reg=nidx, elem_size=D,
                        queue_num=_nextq(),
                    )
                nc.vector.tensor_tensor(out=h[:], in0=h[:], in1=eg[:], op=OP.add)

            # zero the kvg pool buffers once (trimmed gathers leave stale
            # bytes behind; first use must not see NaN bit patterns)
            for _ in range(EPBUFS):
                z = ep.tile([P, trun, 2 * D], dt.bfloat16, tag="kvg")
                nc.vector.memset(z[:], 0.0)

            # ---- pre-pass: LN + projections for layer 0
            for t in range(NBLK):
                _lnproj_block(t, 0)

            # ---- layers ----------------------------------------------------
            for layer in range(L):
                if PROBE_NO_COLLECTIVE:
                    nc.gpsimd.dma_start(out=kvf.ap()[0:NPAD, :], in_=kvb.ap())
                else:
                    nc.gpsimd.collective_compute(
                        "AllGather",
                        OP.bypass,
                        replica_groups=[list(range(NCORES))],
                        ins=[kvb.ap().opt()],
                        outs=[kvf.ap().opt()],
                    )

                # ---- edge phase: per (tgt block, bucket) run of trun tiles
                for blk in range(NBLK):
                    pm = psC.tile([P, D + H], dt.float32, tag="pm")
                    for b in range(2):
                        off = (blk * 2 + b) * EDB
                        ed = ep.tile([P, EDB], dt.uint8, tag="ed")
                        nc.sync.dma_start(ed[:], ed_p.ap()[:, off : off + EDB])
                        ki = ed[:, 0:KIB].bitcast(dt.int16)         # [P, trun*8]
                        sgv = ed[:, KIB : KIB + nrun].bitcast(dt.float8e4).rearrange(
                            "p (t e) -> p t e", e=P
                        )
                        stv = ed[:, KIB + nrun : EDB].bitcast(dt.float8e4).rearrange(
                            "p (t e) -> p t e", e=P
                        )

                        if ABL_NOEDGE:
                            ue0 = ep.tile([P, trun, D + H], dt.bfloat16, tag="ue")
                            nc.vector.memset(ue0[:], 0.5)
                            for tt in range(trun):
                                nc.tensor.matmul(
                                    out=pm[:], lhsT=sgv[:, tt, :], rhs=ue0[:, tt, :],
                                    start=(b == 0 and tt == 0),
                                    stop=(b == 1 and tt == trun - 1),
                                )
                            continue
                        kvg = ep.tile([P, trun, 2 * D], dt.bfloat16, tag="kvg")
                        for i0 in range(0, trun, GBATCH):
                            i1 = min(i0 + GBATCH, trun)
                            nidx = (i1 - i0) * P
                            nc.gpsimd.dma_gather(
                                out_ap=kvg[:, i0:i1, :],
                                in_ap=kvf.ap()[b * NB_ROWS : (b + 1) * NB_ROWS, :],
                                idxs_ap=ki[:, i0 * 8 : i1 * 8],
                                num_idxs=nidx, num_idxs_reg=nidx,
                                elem_size=2 * D,
                                queue_num=_nextq(),
                            )
                        # q-broadcast via PE: qg[e, f] = q[tl(e), f]
                        qg = ep.tile([P, trun, D], dt.bfloat16, tag="qg")
                        for c0 in range(0, nrun, 512):
                            c1 = min(c0 + 512, nrun)
                            qp = psQ.tile([P, 512], dt.float32, tag="qp")
                            for tt in range(c0 // P, c1 // P):
                                o = tt * P - c0
                                nc.tensor.matmul(
                                    out=qp[:, o : o + P], lhsT=stv[:, tt, :],
                                    rhs=qsb[:, blk, :], start=True, stop=True,
                                )
                            nc.scalar.copy(
                                qg[:, c0 // P : c1 // P, :],
                                qp[:, 0 : c1 - c0].rearrange("p (t e) -> p t e", e=P),
                            )
                        # per-edge logits: alpha = sum_c q*k (tree reduce)
                        qk = ep.tile([P, trun, H, C], dt.bfloat16, tag="qk")
                        nc.vector.tensor_tensor(
                            out=qk[:].rearrange("p t h c -> p t (h c)"),
                            in0=qg[:], in1=kvg[:, :, 0:D], op=OP.mult,
                        )
                        t1 = ep.tile([P, trun, H, 4], dt.bfloat16, tag="t1")
                        with nc.allow_low_precision(reason="alpha logits are O(0.1)"):
                            nc.vector.tensor_tensor(
                                out=t1[:], in0=qk[:, :, :, 0:4], in1=qk[:, :, :, 4:8],
                                op=OP.add,
                            )
                            t2 = ep.tile([P, trun, H, 2], dt.bfloat16, tag="t2")
                            nc.vector.tensor_tensor(
                                out=t2[:], in0=t1[:, :, :, 0:2], in1=t1[:, :, :, 2:4],
                                op=OP.add,
                            )
                            al = ep.tile([P, trun, H, 1], dt.bfloat16, tag="al")
                            nc.vector.tensor_tensor(
                                out=al[:], in0=t2[:, :, :, 0:1], in1=t2[:, :, :, 1:2],
                                op=OP.add,
                            )
                        ue = ep.tile([P, trun, D + H], dt.bfloat16, tag="ue")
                        nc.scalar.activation(
                            out=ue[:, :, D : D + H].rearrange("p t (h o) -> p t h o", o=1),
                            in_=al[:], func=AF.Exp,
                        )
                        wex = ep.tile([P, trun, H, C], dt.bfloat16, tag="wex")
                        nc.scalar.activation(
                            out=wex[:], in_=al[:].to_broadcast([P, trun, H, C]),
                            func=AF.Exp,
                        )
                        nc.vector.tensor_tensor(
                            out=ue[:, :, 0:D], in0=kvg[:, :, D : 2 * D],
                            in1=wex[:].rearrange("p t h c -> p t (h c)"), op=OP.mult,
                        )
                        for tt in range(trun):
                            nc.tensor.matmul(
                                out=pm[:], lhsT=sgv[:, tt, :], rhs=ue[:, tt, :],
                                start=(b == 0 and tt == 0),
                                stop=(b == 1 and tt == trun - 1),
                            )

                    # ---- fused per-block tail: normalize, gate, residual,
                    # then next layer's LN+projection (or final LN) ----------
                    msgb = sp.tile([P, D], dt.float32, tag="msgb")
                    rden = sp.tile([P, H, 1], dt.float32, tag="rden")
                    nc.vector.tensor_scalar_add(
                        rden[:], pm[:, D : D + H].rearrange("p (h o) -> p h o", o=1), 1e-20
                    )
                    nc.vector.reciprocal(rden[:], rden[:])
                    nc.vector.tensor_tensor(
                        out=msgb[:].rearrange("p (h c) -> p h c", c=C),
                        in0=pm[:, 0:D].rearrange("p (h c) -> p h c", c=C),
                        in1=rden[:].to_broadcast([P, H, C]),
                        op=OP.mult,
                    )
                    nc.vector.tensor_tensor(
                        out=msgb[:], in0=msgb[:], in1=bvt[:, layer, :], op=OP.add
                    )
                    scr = sp.tile([P, D], dt.float32, tag="scr")
                    bs1 = sp.tile([P, 1], dt.float32, tag="bs1")
                    nc.vector.scalar_tensor_tensor(
                        out=scr[:], in0=msgb[:], scalar=1.0, in1=wb[:, layer, 0:D],
                        op0=OP.mult, op1=OP.mult, accum_out=bs1[:],
                    )
                    bs2 = sp.tile([P, 1], dt.float32, tag="bs2")
                    nc.vector.scalar_tensor_tensor(
                        out=scr[:], in0=xr[:, blk, :], scalar=1.0, in1=wb[:, layer, D : 2 * D],
                        op0=OP.mult, op1=OP.mult, accum_out=bs2[:],
                    )
                    nc.vector.tensor_tensor(out=bs1[:], in0=bs1[:], in1=bs2[:], op=OP.add)
                    beta = sp.tile([P, 1], dt.float32, tag="beta")
                    nc.scalar.activation(out=beta[:], in_=bs1[:], func=AF.Exp, scale=-1.0)
                    nc.vector.tensor_scalar_add(beta[:], beta[:], 1.0)
                    nc.vector.reciprocal(beta[:], beta[:])
                    # h += msg + beta*(xr - msg)
                    tmpb = sp.tile([P, D], dt.float32, tag="tmpb")
                    nc.vector.tensor_tensor(
                        out=tmpb[:], in0=xr[:, blk, :], in1=msgb[:], op=OP.subtract
                    )
                    nc.vector.scalar_tensor_tensor(
                        out=tmpb[:], in0=tmpb[:], scalar=beta[:], in1=msgb[:],
                        op0=OP.mult, op1=OP.add,
                    )
                    nc.vector.tensor_tensor(
                        out=h[:, blk, :], in0=h[:, blk, :], in1=tmpb[:], op=OP.add
                    )
                    if layer == L - 1:
                        _final_block(blk)
                    else:
                        _lnproj_block(blk, layer + 1)

    nc.finalize()
    return nc

LAST_RES = None


def _make_in_maps(inputs, cores):
    sq = 1.0 / np.sqrt(np.float32(C))
    Wq, Wk, Wv, Wsk = (np.asarray(inputs[k], dtype=np.float32) for k in ("Wq", "Wk", "Wv", "Wskip"))
    bq, bv, bsk = (np.asarray(inputs[k], dtype=np.float32) for k in ("bq", "bv", "bskip"))
    # order per layer: k | v | q*sq | skip  (k-bias dropped: softmax shift
    # invariance; v-bias folded in post-aggregation)
    wcat = np.concatenate([Wk, Wv, Wq * sq, Wsk], axis=2).transpose(1, 0, 2).reshape(D, L * 4 * D)
    bcat = np.concatenate([bq * sq, bsk], axis=1)  # [L, 2D]
    bcat_rep = np.broadcast_to(bcat[:, None, :], (L, P, 2 * D)).transpose(1, 0, 2).reshape(P, L * 2 * D).copy()
    bvp = np.broadcast_to(bv[:, None, :], (L, P, D)).transpose(1, 0, 2).reshape(P, L * D).copy()
    lns, lnb = np.asarray(inputs["ln_scale"], np.float32), np.asarray(inputs["ln_bias"], np.float32)
    lnp = np.broadcast_to(
        np.concatenate([lns, lnb], axis=1)[:, None, :], (L, P, 2 * D)
    ).transpose(1, 0, 2).reshape(P, L * 2 * D).copy()
    fnp = np.broadcast_to(
        np.concatenate([inputs["fn_scale"], inputs["fn_bias"]])[None, :], (P, 2 * D)
    ).astype(np.float32).copy()
    Wbeta = np.asarray(inputs["Wbeta"], np.float32)  # [L, 3D, 1]
    wa = Wbeta[:, 0:D, 0] + Wbeta[:, 2 * D : 3 * D, 0]      # msg coeff
    wbx = Wbeta[:, D : 2 * D, 0] - Wbeta[:, 2 * D : 3 * D, 0]  # xr coeff
    wbeta_rep = np.broadcast_to(
        np.concatenate([wa, wbx], axis=1)[:, None, :], (L, P, 2 * D)
    ).transpose(1, 0, 2).reshape(P, L * 2 * D).copy()
    bin_rep = np.broadcast_to(
        np.asarray(inputs["b_in"], np.float32)[None, :], (P, D)
    ).copy()

    common = dict(
        emb_in=np.asarray(inputs["in_emb"], np.float32),
        emb_out=np.asarray(inputs["out_emb"], np.float32),
        win=_bf16(inputs["W_in"]),
        bin=bin_rep,
        wcat=_bf16(wcat),
        bcat=bcat_rep,
        bvp=bvp,
        lnp=lnp,
        fnp=fnp,
        wbeta=wbeta_rep,
    )
    in_maps = []
    for c in range(NCORES):
        m = dict(common)
        cd = cores[c]
        m.update(x=cd["x"], idg=cd["idg"], odg=cd["odg"], edata=cd["edata"])
        in_maps.append(m)
    return in_maps


def kernel(**inputs):
    import os

    from concourse.bass_utils import run_bass_kernel_spmd

    x = np.asarray(inputs["x"], dtype=np.float32)
    edge_index = np.asarray(inputs["edge_index"])
    cores, trun, EDB = _preprocess(x, edge_index)
    in_maps = _make_in_maps(inputs, cores)

    nc = _build(trun, EDB)
    kw = {}
    td = os.environ.get("BASS_KERNEL_TMPDIR")
    if td:
        kw["tmpdir"] = td
    res = run_bass_kernel_spmd(nc, in_maps, core_ids=list(range(NCORES)), **kw)
    global LAST_RES
    LAST_RES = res
    outs = [np.asarray(r["out"], dtype=np.float32) for r in res.results]
    return np.concatenate(outs, axis=0)


if __name__ == "__main__":
    import reference

    inp = {k: np.asarray(v) for k, v in reference.setup_inputs().items()}
    exp = np.asarray(reference.reference(**inp))
    act = kernel(**inp)
    err = np.abs(act - exp).max() / (np.abs(exp).max() + 1e-9)
    print("Relative error:", err)


# revision 16
# speedup vs baseline: 1.8192x; 1.1889x over previous
"""Graphormer-expert GNN kernel for 8 Trainium2 NeuronCores.

Strategy (matches the sharding hint): nodes are partitioned 8 x 6250 (graph
parallel); each core owns the edges whose *target* falls in its shard, so the
scatter-softmax is core-local.  Per layer each core computes LN + projections
for its own nodes, k|v rows (bf16) are exchanged with an AllGather, and
per-edge k/v rows are fetched with SWDGE dma_gather (int16 indices; source
table split in two 25088-row buckets; padding indices are -1 so the Q7
descriptor generator trims them).  Per-edge softmax runs without
max-subtraction (|alpha| << 1 for this model); the q-row broadcast and the
segment sums (softmax denominator + message aggregation) run on the
TensorEngine with host-precomputed fp8 one-hot matrices (exact), accumulating
each target-block's edge tiles in PSUM.  The softmax division is commuted
past the segment sum and applied per node.  k-bias drops out exactly (softmax
shift invariance); v-bias is folded in after aggregation.  ACT runs only
copies + Exp (sigmoid via exp, layernorm rsqrt via DVE Newton iteration) so
the activation table is loaded once.
"""

import sys

sys.path.insert(0, "/opt/trn_rl_repo")

import numpy as np

N, IN_DIM, D, H, L, E, MAX_DEG = 50000, 128, 128, 16, 3, 800000, 512
C = D // H
P = 128
NCORES = 8
NSH = N // NCORES            # 6250 nodes per core
NBLK = (NSH + P - 1) // P    # 49 target blocks per core
NPAD = NBLK * P              # 6272 padded rows per core
NB_ROWS = NCORES * NPAD // 2  # 25088 rows per src bucket (fits int16)

RSQRT_MAGIC = 0x5F3759DF


def _bf16(a):
    import ml_dtypes

    return np.asarray(a, dtype=ml_dtypes.bfloat16)


def _fp8(a):
    import ml_dtypes

    return np.asarray(a, dtype=ml_dtypes.float8_e4m3)


def _wrap_idx16(idx, pad_to=None, fill=0):
    """int16 idx array -> [128, n/16] wrapped (j -> [j%16, j//16]) and
    replicated across the 8 gpsimd cores' 16-partition groups."""
    n = len(idx) if pad_to is None else pad_to
    assert n % 16 == 0
    a = np.full(n, fill, dtype=np.int16)
    a[: len(idx)] = idx.astype(np.int16)
    w = a.reshape(n // 16, 16).T  # [16, n/16]
    return np.tile(w, (8, 1))  # [128, n/16]


def _preprocess(x, edge_index):
    """Host-side integer/index preprocessing + per-core shard arrays."""
    src = np.asarray(edge_index[0], dtype=np.int64)
    tgt = np.asarray(edge_index[1], dtype=np.int64)

    # degrees (int) for the centrality-embedding gather
    idg = np.clip(np.bincount(tgt, minlength=N), 0, MAX_DEG)
    odg = np.clip(np.bincount(src, minlength=N), 0, MAX_DEG)

    # global row in the AllGather'ed kv table of node g
    kv_row = (src // NSH) * NPAD + (src % NSH)
    bucket = (kv_row >= NB_ROWS).astype(np.int64)
    src_loc = kv_row - bucket * NB_ROWS  # 0..25087, int16-safe

    # first pass: find the max (block,bucket) run length across all cores
    run_max = 0
    per_core = []
    for c in range(NCORES):
        m = (tgt // NSH) == c
        cs, ct, cb, csl = src[m], tgt[m] - c * NSH, bucket[m], src_loc[m]
        blk = ct // P
        cnt = np.bincount(blk * 2 + cb, minlength=NBLK * 2)
        run_max = max(run_max, cnt.max())
        per_core.append((cs, ct, cb, csl, blk))
    trun = int((run_max + P - 1) // P)  # tiles per (block,bucket) run
    nrun = trun * P
    KIB = trun * 16                     # idx bytes per partition per run
    EDB = KIB + 2 * trun * P            # + sg fp8 + st fp8 bytes

    ncalls = (trun + GBATCH - 1) // GBATCH
    cores = []
    for c in range(NCORES):
        cs, ct, cb, csl, blk = per_core[c]
        order = np.lexsort((ct, cb, blk))
        cs, ct, cb, csl, blk = (a[order] for a in (cs, ct, cb, csl, blk))

        edata = np.zeros((P, NBLK * 2 * EDB), dtype=np.uint8)
        gcnt = np.zeros((NBLK * 2, ncalls), dtype=np.int32)
        for k in range(NBLK):
            for b in range(2):
                sel = (cb == b) & (blk == k)
                n_e = int(sel.sum())
                for ci, i0 in enumerate(range(0, trun, GBATCH)):
                    i1 = min(i0 + GBATCH, trun)
                    gcnt[k * 2 + b, ci] = max(0, min(n_e - i0 * P, (i1 - i0) * P))
                ki = _wrap_idx16(csl[sel], pad_to=nrun, fill=-1)  # [128, trun*8]
                tl = ct[sel] - k * P  # 0..127 col within the block
                ee = np.arange(n_e)
                S = np.zeros((P, nrun), dtype=np.float32)   # [e%128, t*128+tl]
                S[ee % P, (ee // P) * P + tl] = 1.0
                ST = np.zeros((P, nrun), dtype=np.float32)  # [tl, e]
                ST[tl, ee] = 1.0
                off = (k * 2 + b) * EDB
                edata[:, off : off + KIB] = ki.view(np.uint8)
                edata[:, off + KIB : off + KIB + nrun] = _fp8(S).view(np.uint8)
                edata[:, off + KIB + nrun : off + EDB] = _fp8(ST).view(np.uint8)

        cores.append(
            dict(
                edata=edata,
                gcnt=np.broadcast_to(gcnt.reshape(1, -1), (P, NBLK * 2 * ncalls)).copy(),
                idg=_wrap_idx16(np.pad(idg[c * NSH : (c + 1) * NSH], (0, NPAD - NSH))),
                odg=_wrap_idx16(np.pad(odg[c * NSH : (c + 1) * NSH], (0, NPAD - NSH))),
                x=np.pad(
                    np.asarray(x[c * NSH : (c + 1) * NSH], dtype=np.float32),
                    ((0, NPAD - NSH), (0, 0)),
                ),
            )
        )
    return cores, trun, EDB


import os as _os

PROBE_NO_COLLECTIVE = bool(int(_os.environ.get("KB_NOCOLL", "0")))
ABL_NOEDGE = bool(int(_os.environ.get("KB_NOEDGE", "0")))   # skip edge interior

GBATCH = int(_os.environ.get("KB_GBATCH", "5"))   # kv gather tiles per call
EBATCH = int(_os.environ.get("KB_EBATCH", "3"))   # emb gather blocks per call
EPBUFS = int(_os.environ.get("KB_EPBUFS", "4"))   # edge pool depth


def _build(trun, EDB):
    from concourse import bass, mybir
    import concourse.tile as tile
    from concourse.bacc import Bacc
    from concourse.masks import make_identity

    dt = mybir.dt
    AX = mybir.AxisListType
    OP = mybir.AluOpType
    AF = mybir.ActivationFunctionType

    KIB = trun * 16
    nrun = trun * P

    nc = Bacc(None, target_bir_lowering=False, debug=False, num_devices=NCORES,
              num_swdge_queues=4)
    qctr = [0]

    def _nextq():
        qctr[0] = (qctr[0] + 1) % 4
        return qctr[0]

    # ---- parameters (per core) -------------------------------------------
    xin = nc.declare_dram_parameter("x", [NPAD, D], dt.float32, isOutput=False)
    emb_i = nc.declare_dram_parameter("emb_in", [MAX_DEG + 1, D], dt.float32, isOutput=False)
    emb_o = nc.declare_dram_parameter("emb_out", [MAX_DEG + 1, D], dt.float32, isOutput=False)
    idg_p = nc.declare_dram_parameter("idg", [P, NPAD // 16], dt.int16, isOutput=False)
    odg_p = nc.declare_dram_parameter("odg", [P, NPAD // 16], dt.int16, isOutput=False)
    win_p = nc.declare_dram_parameter("win", [D, D], dt.bfloat16, isOutput=False)
    bin_p = nc.declare_dram_parameter("bin", [P, D], dt.float32, isOutput=False)
    wcat_p = nc.declare_dram_parameter("wcat", [D, L * 4 * D], dt.bfloat16, isOutput=False)
    bcat_p = nc.declare_dram_parameter("bcat", [P, L * 2 * D], dt.float32, isOutput=False)
    bvp_p = nc.declare_dram_parameter("bvp", [P, L * D], dt.float32, isOutput=False)
    lnp_p = nc.declare_dram_parameter("lnp", [P, L * 2 * D], dt.float32, isOutput=False)
    fnp_p = nc.declare_dram_parameter("fnp", [P, 2 * D], dt.float32, isOutput=False)
    wb_p = nc.declare_dram_parameter("wbeta", [P, L * 2 * D], dt.float32, isOutput=False)
    ed_p = nc.declare_dram_parameter("edata", [P, NBLK * 2 * EDB], dt.uint8, isOutput=False)
    NC_G = (trun + GBATCH - 1) // GBATCH
    gc_p = nc.declare_dram_parameter("gcnt", [P, NBLK * 2 * NC_G], dt.int32, isOutput=False)
    out_p = nc.declare_dram_parameter("out", [NSH, D], dt.float32, isOutput=True)

    # ---- DRAM scratch -----------------------------------------------------
    kvb = nc.dram_tensor("kv_bounce", [NPAD, 2 * D], dt.bfloat16)
    kvf = nc.dram_tensor("kv_full", [NCORES * NPAD, 2 * D], dt.bfloat16, addr_space="Shared")

    with tile.TileContext(nc) as tc:
        with (
            tc.tile_pool(name="persist", bufs=1) as pp,
            tc.tile_pool(name="wtiles", bufs=1) as wp,
            tc.tile_pool(name="work", bufs=1) as kp,
            tc.tile_pool(name="small", bufs=3) as sp,
            tc.tile_pool(name="edge", bufs=EPBUFS) as ep,
            tc.tile_pool(name="psA", bufs=1, space="PSUM") as psA,
            tc.tile_pool(name="psB", bufs=1, space="PSUM") as psB,
            tc.tile_pool(name="psC", bufs=2, space="PSUM") as psC,
            tc.tile_pool(name="psQ", bufs=2, space="PSUM") as psQ,
        ):
            # persistent state
            h = pp.tile([P, NBLK, D], dt.float32, tag="h")
            xr = pp.tile([P, NBLK, D], dt.bfloat16, tag="xr")
            qsb = pp.tile([P, NBLK, D], dt.bfloat16, tag="qsb")

            ident = wp.tile([P, P], dt.bfloat16, tag="ident")
            make_identity(nc, ident[:])
            win = wp.tile([D, D], dt.bfloat16, tag="win")
            nc.sync.dma_start(win[:], win_p.ap())
            bin_t = wp.tile([P, D], dt.float32, tag="bin")
            nc.sync.dma_start(bin_t[:], bin_p.ap())
            wcat = wp.tile([D, L, 4 * D], dt.bfloat16, tag="wcat")
            nc.sync.dma_start(wcat[:], wcat_p.ap())
            bcat = wp.tile([P, L, 2 * D], dt.float32, tag="bcat")
            nc.sync.dma_start(bcat[:], bcat_p.ap())
            bvt = wp.tile([P, L, D], dt.float32, tag="bvt")
            nc.sync.dma_start(bvt[:], bvp_p.ap())
            lnp = wp.tile([P, L, 2 * D], dt.float32, tag="lnp")
            nc.sync.dma_start(lnp[:], lnp_p.ap())
            fnp = wp.tile([P, 2 * D], dt.float32, tag="fnp")
            nc.sync.dma_start(fnp[:], fnp_p.ap())
            wb = wp.tile([P, L, 2 * D], dt.float32, tag="wb")
            nc.sync.dma_start(wb[:], wb_p.ap())
            gct = wp.tile([P, NBLK * 2 * NC_G], dt.int32, tag="gct")
            nc.sync.dma_start(gct[:], gc_p.ap())
            greg = nc.gpsimd.alloc_register("gcnt_reg")

            def _rsqrt(rs, ve):
                """rs = 1/sqrt(ve) via bit-hack seed + 2 Newton iterations.
                rs, ve: [P, 1] f32 tiles (DVE only — no ACT table)."""
                iv = sp.tile([P, 1], dt.int32, tag="nw_i")
                nc.vector.tensor_scalar(
                    out=iv[:], in0=ve[:].bitcast(dt.int32), scalar1=1,
                    scalar2=None, op0=OP.logical_shift_right,
                )
                nc.vector.tensor_scalar(
                    out=iv[:], in0=iv[:], scalar1=-1, scalar2=RSQRT_MAGIC,
                    op0=OP.mult, op1=OP.add,
                )
                y = iv[:].bitcast(dt.float32)
                t = sp.tile([P, 1], dt.float32, tag="nw_t")
                for _ in range(2):
                    nc.vector.tensor_tensor(out=t[:], in0=y, in1=y, op=OP.mult)
                    nc.vector.tensor_tensor(out=t[:], in0=t[:], in1=ve[:], op=OP.mult)
                    nc.vector.tensor_scalar(
                        out=t[:], in0=t[:], scalar1=-0.5, scalar2=1.5,
                        op0=OP.mult, op1=OP.add,
                    )
                    nc.vector.tensor_tensor(out=t[:], in0=y, in1=t[:], op=OP.mult)
                    nc.vector.tensor_copy(iv[:].bitcast(dt.float32), t[:])
                nc.vector.tensor_copy(rs[:], y)

            def _ln_to(hb, t, scale_ap, bias_ap):
                """hb[P, D] (bf16) = LN(h[:, t, :]) * scale + bias."""
                stats = sp.tile([P, 6], dt.float32, tag="bst")
                nc.vector.bn_stats(stats[:], h[:, t, :])
                mv = sp.tile([P, 2], dt.float32, tag="mv")
                nc.vector.bn_aggr(mv[:], stats[:])
                ve = sp.tile([P, 1], dt.float32, tag="ve")
                nc.vector.tensor_scalar_add(ve[:], mv[:, 1:2], 1e-5)
                rs = sp.tile([P, 1], dt.float32, tag="rs")
                _rsqrt(rs, ve)
                hf = sp.tile([P, D], dt.float32, tag="hf")
                nc.vector.tensor_tensor(
                    out=hf[:], in0=h[:, t, :],
                    in1=mv[:, 0:1].to_broadcast([P, D]), op=OP.subtract,
                )
                nc.vector.scalar_tensor_tensor(
                    out=hb[:], in0=hf[:], scalar=rs[:], in1=scale_ap,
                    op0=OP.mult, op1=OP.mult,
                )
                nc.vector.tensor_tensor(out=hb[:], in0=hb[:], in1=bias_ap, op=OP.add)

            def _lnproj_block(t, layer):
                """LN h[:,t] (lnp[layer]) -> proj (wcat[layer]) -> kvb/qsb/xr."""
                hb = sp.tile([P, D], dt.bfloat16, tag="hb")
                _ln_to(hb, t, lnp[:, layer, 0:D], lnp[:, layer, D : 2 * D])
                pT = psA.tile([P, P], dt.bfloat16, tag="pT")
                nc.tensor.transpose(out=pT[:], in_=hb[:], identity=ident[:])
                hnTt = sp.tile([P, D], dt.bfloat16, tag="hnTt")
                nc.scalar.copy(hnTt[:], pT[:])
                ps = psB.tile([P, 4 * D], dt.float32, tag="ps")
                nc.tensor.matmul(
                    out=ps[:], lhsT=hnTt[:], rhs=wcat[:, layer, :],
                    start=True, stop=True,
                )
                kvq = sp.tile([P, 2 * D], dt.bfloat16, tag="kvq")
                nc.scalar.copy(kvq[:], ps[:, 0 : 2 * D])
                nc.vector.scalar_tensor_tensor(
                    out=qsb[:, t, :], in0=ps[:, 2 * D : 3 * D], scalar=1.0,
                    in1=bcat[:, layer, 0:D], op0=OP.mult, op1=OP.add,
                )
                nc.vector.scalar_tensor_tensor(
                    out=xr[:, t, :], in0=ps[:, 3 * D : 4 * D], scalar=1.0,
                    in1=bcat[:, layer, D : 2 * D], op0=OP.mult, op1=OP.add,
                )
                nc.sync.dma_start(kvb.ap()[t * P : (t + 1) * P, :], kvq[:])

            def _final_block(t):
                """Final LN on h[:,t] -> out DMA."""
                ot = sp.tile([P, D], dt.float32, tag="ot")
                stats = sp.tile([P, 6], dt.float32, tag="bst")
                nc.vector.bn_stats(stats[:], h[:, t, :])
                mv = sp.tile([P, 2], dt.float32, tag="mv")
                nc.vector.bn_aggr(mv[:], stats[:])
                ve = sp.tile([P, 1], dt.float32, tag="ve")
                nc.vector.tensor_scalar_add(ve[:], mv[:, 1:2], 1e-5)
                rs = sp.tile([P, 1], dt.float32, tag="rs")
                _rsqrt(rs, ve)
                nc.vector.tensor_tensor(
                    out=ot[:], in0=h[:, t, :],
                    in1=mv[:, 0:1].to_broadcast([P, D]), op=OP.subtract,
                )
                nc.vector.scalar_tensor_tensor(
                    out=ot[:], in0=ot[:], scalar=rs[:], in1=fnp[:, 0:D],
                    op0=OP.mult, op1=OP.mult,
                )
                nc.vector.tensor_tensor(
                    out=ot[:], in0=ot[:], in1=fnp[:, D : 2 * D], op=OP.add
                )
                lo = t * P
                hi = min((t + 1) * P, NSH)
                if hi > lo:
                    nc.sync.dma_start(out_p.ap()[lo:hi, :], ot[0 : hi - lo, :])

            # ---- phase 0: h = x @ W_in + b_in + emb_in[idg] + emb_out[odg]
            for t in range(NBLK):
                xt = sp.tile([P, D], dt.float32, tag="xt")
                nc.sync.dma_start(xt[:], xin.ap()[t * P : (t + 1) * P, :])
                xb = sp.tile([P, D], dt.bfloat16, tag="xb")
                nc.vector.tensor_copy(xb[:], xt[:])
                pT = psA.tile([P, P], dt.bfloat16, tag="pT")
                nc.tensor.transpose(out=pT[:], in_=xb[:], identity=ident[:])
                xTb = sp.tile([P, D], dt.bfloat16, tag="xTb")
                nc.scalar.copy(xTb[:], pT[:])
                ph = psB.tile([P, D], dt.float32, tag="ph")
                nc.tensor.matmul(out=ph[:], lhsT=xTb[:], rhs=win[:], start=True, stop=True)
                nc.vector.scalar_tensor_tensor(
                    out=h[:, t, :], in0=ph[:], scalar=1.0, in1=bin_t[:],
                    op0=OP.mult, op1=OP.add,
                )
            for tabl, idxp in ((emb_i, idg_p), (emb_o, odg_p)):
                gi = kp.tile([P, NPAD // 16], dt.int16, tag="gidx")
                nc.sync.dma_start(gi[:], idxp.ap())
                eg = kp.tile([P, NBLK, D], dt.float32, tag="scratch")
                for i0 in range(0, NBLK, EBATCH):
                    i1 = min(i0 + EBATCH, NBLK)
                    nidx = (i1 - i0) * P
                    nc.gpsimd.dma_gather(
                        out_ap=eg[:, i0:i1, :], in_ap=tabl.ap(),
                        idxs_ap=gi[:, i0 * 8 : i1 * 8],
                        num_idxs=nidx, num_idxs_reg=nidx, elem_size=D,
                        queue_num=_nextq(),
                    )
                nc.vector.tensor_tensor(out=h[:], in0=h[:], in1=eg[:], op=OP.add)

            # zero the kvg pool buffers once (trimmed gathers leave stale
            # bytes behind; first use must not see NaN bit patterns)
            for _ in range(EPBUFS):
                z = ep.tile([P, trun, 2 * D], dt.bfloat16, tag="kvg")
                nc.vector.memset(z[:], 0.0)

            # ---- pre-pass: LN + projections for layer 0
            for t in range(NBLK):
                _lnproj_block(t, 0)

            # ---- layers ----------------------------------------------------
            for layer in range(L):
                if PROBE_NO_COLLECTIVE:
                    nc.gpsimd.dma_start(out=kvf.ap()[0:NPAD, :], in_=kvb.ap())
                else:
                    nc.gpsimd.collective_compute(
                        "AllGather",
                        OP.bypass,
                        replica_groups=[list(range(NCORES))],
                        ins=[kvb.ap().opt()],
                        outs=[kvf.ap().opt()],
                    )

                # ---- edge phase: per (tgt block, bucket) run of trun tiles
                for blk in range(NBLK):
                    pm = psC.tile([P, D + H], dt.float32, tag="pm")
                    for b in range(2):
                        off = (blk * 2 + b) * EDB
                        ed = ep.tile([P, EDB], dt.uint8, tag="ed")
                        nc.sync.dma_start(ed[:], ed_p.ap()[:, off : off + EDB])
                        ki = ed[:, 0:KIB].bitcast(dt.int16)         # [P, trun*8]
                        sgv = ed[:, KIB : KIB + nrun].bitcast(dt.float8e4).rearrange(
                            "p (t e) -> p t e", e=P
                        )
                        stv = ed[:, KIB + nrun : EDB].bitcast(dt.float8e4).rearrange(
                            "p (t e) -> p t e", e=P
                        )

                        if ABL_NOEDGE:
                            ue0 = ep.tile([P, trun, D + H], dt.bfloat16, tag="ue")
                            nc.vector.memset(ue0[:], 0.5)
                            for tt in range(trun):
                                nc.tensor.matmul(
                                    out=pm[:], lhsT=sgv[:, tt, :], rhs=ue0[:, tt, :],
                                    start=(b == 0 and tt == 0),
                                    stop=(b == 1 and tt == trun - 1),
                                )
                            continue
                        kvg = ep.tile([P, trun, 2 * D], dt.bfloat16, tag="kvg")
                        for ci, i0 in enumerate(range(0, trun, GBATCH)):
                            i1 = min(i0 + GBATCH, trun)
                            nidx = (i1 - i0) * P
                            gj = (blk * 2 + b) * NC_G + ci
                            nc.gpsimd.reg_load(greg, gct[0:1, gj : gj + 1])
                            nc.gpsimd.dma_gather(
                                out_ap=kvg[:, i0:i1, :],
                                in_ap=kvf.ap()[b * NB_ROWS : (b + 1) * NB_ROWS, :],
                                idxs_ap=ki[:, i0 * 8 : i1 * 8],
                                num_idxs=nidx, num_idxs_reg=greg,
                                elem_size=2 * D,
                                queue_num=_nextq(),
                            )
                        # q-broadcast via PE: qg[e, f] = q[tl(e), f]
                        qg = ep.tile([P, trun, D], dt.bfloat16, tag="qg")
                        for c0 in range(0, nrun, 512):
                            c1 = min(c0 + 512, nrun)
                            qp = psQ.tile([P, 512], dt.float32, tag="qp")
                            for tt in range(c0 // P, c1 // P):
                                o = tt * P - c0
                                nc.tensor.matmul(
                                    out=qp[:, o : o + P], lhsT=stv[:, tt, :],
                                    rhs=qsb[:, blk, :], start=True, stop=True,
                                )
                            nc.scalar.copy(
                                qg[:, c0 // P : c1 // P, :],
                                qp[:, 0 : c1 - c0].rearrange("p (t e) -> p t e", e=P),
                            )
                        # per-edge logits: alpha = sum_c q*k (tree reduce)
                        qk = ep.tile([P, trun, H, C], dt.bfloat16, tag="qk")
                        nc.vector.tensor_tensor(
                            out=qk[:].rearrange("p t h c -> p t (h c)"),
                            in0=qg[:], in1=kvg[:, :, 0:D], op=OP.mult,
                        )
                        t1 = ep.tile([P, trun, H, 4], dt.bfloat16, tag="t1")
                        with nc.allow_low_precision(reason="alpha logits are O(0.1)"):
                            nc.vector.tensor_tensor(
                                out=t1[:], in0=qk[:, :, :, 0:4], in1=qk[:, :, :, 4:8],
                                op=OP.add,
                            )
                            t2 = ep.tile([P, trun, H, 2], dt.bfloat16, tag="t2")
                            nc.vector.tensor_tensor(
                                out=t2[:], in0=t1[:, :, :, 0:2], in1=t1[:, :, :, 2:4],
                                op=OP.add,
                            )
                            al = ep.tile([P, trun, H, 1], dt.bfloat16, tag="al")
                            nc.vector.tensor_tensor(
                                out=al[:], in0=t2[:, :, :, 0:1], in1=t2[:, :, :, 1:2],
                                op=OP.add,
                            )
                        ue = ep.tile([P, trun, D + H], dt.bfloat16, tag="ue")
                        nc.scalar.activation(
                            out=ue[:, :, D : D + H].rearrange("p t (h o) -> p t h o", o=1),
                            in_=al[:], func=AF.Exp,
                        )
                        wex = ep.tile([P, trun, H, C], dt.bfloat16, tag="wex")
                        nc.scalar.activation(
                            out=wex[:], in_=al[:].to_broadcast([P, trun, H, C]),
                            func=AF.Exp,
                        )
                        nc.vector.tensor_tensor(
                            out=ue[:, :, 0:D], in0=kvg[:, :, D : 2 * D],
                            in1=wex[:].rearrange("p t h c -> p t (h c)"), op=OP.mult,
                        )
                        for tt in range(trun):
                            nc.tensor.matmul(
                                out=pm[:], lhsT=sgv[:, tt, :], rhs=ue[:, tt, :],
                                start=(b == 0 and tt == 0),
                                stop=(b == 1 and tt == trun - 1),
                            )

                    # ---- fused per-block tail: normalize, gate, residual,
                    # then next layer's LN+projection (or final LN) ----------
                    msgb = sp.tile([P, D], dt.float32, tag="msgb")
                    rden = sp.tile([P, H, 1], dt.float32, tag="rden")
                    nc.vector.tensor_scalar_add(
                        rden[:], pm[:, D : D + H].rearrange("p (h o) -> p h o", o=1), 1e-20
                    )
                    nc.vector.reciprocal(rden[:], rden[:])
                    nc.vector.tensor_tensor(
                        out=msgb[:].rearrange("p (h c) -> p h c", c=C),
                        in0=pm[:, 0:D].rearrange("p (h c) -> p h c", c=C),
                        in1=rden[:].to_broadcast([P, H, C]),
                        op=OP.mult,
                    )
                    nc.vector.tensor_tensor(
                        out=msgb[:], in0=msgb[:], in1=bvt[:, layer, :], op=OP.add
                    )
                    scr = sp.tile([P, D], dt.float32, tag="scr")
                    bs1 = sp.tile([P, 1], dt.float32, tag="bs1")
                    nc.vector.scalar_tensor_tensor(
                        out=scr[:], in0=msgb[:], scalar=1.0, in1=wb[:, layer, 0:D],
                        op0=OP.mult, op1=OP.mult, accum_out=bs1[:],
                    )
                    bs2 = sp.tile([P, 1], dt.float32, tag="bs2")
                    nc.vector.scalar_tensor_tensor(
                        out=scr[:], in0=xr[:, blk, :], scalar=1.0, in1=wb[:, layer, D : 2 * D],
                        op0=OP.mult, op1=OP.mult, accum_out=bs2[:],
                    )
                    nc.vector.tensor_tensor(out=bs1[:], in0=bs1[:], in1=bs2[:], op=OP.add)
                    beta = sp.tile([P, 1], dt.float32, tag="beta")
                    nc.scalar.activation(out=beta[:], in_=bs1[:], func=AF.Exp, scale=-1.0)
                    nc.vector.tensor_scalar_add(beta[:], beta[:], 1.0)
                    nc.vector.reciprocal(beta[:], beta[:])
                    # h += msg + beta*(xr - msg)
                    tmpb = sp.tile([P, D], dt.float32, tag="tmpb")
                    nc.vector.tensor_tensor(
                        out=tmpb[:], in0=xr[:, blk, :], in1=msgb[:], op=OP.subtract
                    )
                    nc.vector.scalar_tensor_tensor(
                        out=tmpb[:], in0=tmpb[:], scalar=beta[:], in1=msgb[:],
                        op0=OP.mult, op1=OP.add,
                    )
                    nc.vector.tensor_tensor(
                        out=h[:, blk, :], in0=h[:, blk, :], in1=tmpb[:], op=OP.add
                    )
                    if layer == L - 1:
                        _final_block(blk)
                    else:
                        _lnproj_block(blk, layer + 1)

    nc.finalize()
    return nc

LAST_RES = None


def _make_in_maps(inputs, cores):
    sq = 1.0 / np.sqrt(np.float32(C))
    Wq, Wk, Wv, Wsk = (np.asarray(inputs[k], dtype=np.float32) for k in ("Wq", "Wk", "Wv", "Wskip"))
    bq, bv, bsk = (np.asarray(inputs[k], dtype=np.float32) for k in ("bq", "bv", "bskip"))
    # order per layer: k | v | q*sq | skip  (k-bias dropped: softmax shift
    # invariance; v-bias folded in post-aggregation)
    wcat = np.concatenate([Wk, Wv, Wq * sq, Wsk], axis=2).transpose(1, 0, 2).reshape(D, L * 4 * D)
    bcat = np.concatenate([bq * sq, bsk], axis=1)  # [L, 2D]
    bcat_rep = np.broadcast_to(bcat[:, None, :], (L, P, 2 * D)).transpose(1, 0, 2).reshape(P, L * 2 * D).copy()
    bvp = np.broadcast_to(bv[:, None, :], (L, P, D)).transpose(1, 0, 2).reshape(P, L * D).copy()
    lns, lnb = np.asarray(inputs["ln_scale"], np.float32), np.asarray(inputs["ln_bias"], np.float32)
    lnp = np.broadcast_to(
        np.concatenate([lns, lnb], axis=1)[:, None, :], (L, P, 2 * D)
    ).transpose(1, 0, 2).reshape(P, L * 2 * D).copy()
    fnp = np.broadcast_to(
        np.concatenate([inputs["fn_scale"], inputs["fn_bias"]])[None, :], (P, 2 * D)
    ).astype(np.float32).copy()
    Wbeta = np.asarray(inputs["Wbeta"], np.float32)  # [L, 3D, 1]
    wa = Wbeta[:, 0:D, 0] + Wbeta[:, 2 * D : 3 * D, 0]      # msg coeff
    wbx = Wbeta[:, D : 2 * D, 0] - Wbeta[:, 2 * D : 3 * D, 0]  # xr coeff
    wbeta_rep = np.broadcast_to(
        np.concatenate([wa, wbx], axis=1)[:, None, :], (L, P, 2 * D)
    ).transpose(1, 0, 2).reshape(P, L * 2 * D).copy()
    bin_rep = np.broadcast_to(
        np.asarray(inputs["b_in"], np.float32)[None, :], (P, D)
    ).copy()

    common = dict(
        emb_in=np.asarray(inputs["in_emb"], np.float32),
        emb_out=np.asarray(inputs["out_emb"], np.float32),
        win=_bf16(inputs["W_in"]),
        bin=bin_rep,
        wcat=_bf16(wcat),
        bcat=bcat_rep,
        bvp=bvp,
        lnp=lnp,
        fnp=fnp,
        wbeta=wbeta_rep,
    )
    in_maps = []
    for c in range(NCORES):
        m = dict(common)
        cd = cores[c]
        m.update(x=cd["x"], idg=cd["idg"], odg=cd["odg"], edata=cd["edata"],
                 gcnt=cd["gcnt"])
        in_maps.append(m)
    return in_maps


def kernel(**inputs):
    import os

    from concourse.bass_utils import run_bass_kernel_spmd

    x = np.asarray(inputs["x"], dtype=np.float32)
    edge_index = np.asarray(inputs["edge_index"])
    cores, trun, EDB = _preprocess(x, edge_index)
    in_maps = _make_in_maps(inputs, cores)

    nc = _build(trun, EDB)
    kw = {}
    td = os.environ.get("BASS_KERNEL_TMPDIR")
    if td:
        kw["tmpdir"] = td
    res = run_bass_kernel_spmd(nc, in_maps, core_ids=list(range(NCORES)), **kw)
    global LAST_RES
    LAST_RES = res
    outs = [np.asarray(r["out"], dtype=np.float32) for r in res.results]
    return np.concatenate(outs, axis=0)


if __name__ == "__main__":
    import reference

    inp = {k: np.asarray(v) for k, v in reference.setup_inputs().items()}
    exp = np.asarray(reference.reference(**inp))
    act = kernel(**inp)
    err = np.abs(act - exp).max() / (np.abs(exp).max() + 1e-9)
    print("Relative error:", err)


# revision 18
# speedup vs baseline: 1.8266x; 1.0041x over previous
"""Graphormer-expert GNN kernel for 8 Trainium2 NeuronCores.

Strategy (matches the sharding hint): nodes are partitioned 8 x 6250 (graph
parallel); each core owns the edges whose *target* falls in its shard, so the
scatter-softmax is core-local.  Per layer each core computes LN + projections
for its own nodes, k|v rows (bf16) are exchanged with an AllGather, and
per-edge k/v rows are fetched with SWDGE dma_gather (int16 indices; source
table split in two 25088-row buckets; padding indices are -1 so the Q7
descriptor generator trims them).  Per-edge softmax runs without
max-subtraction (|alpha| << 1 for this model); the q-row broadcast and the
segment sums (softmax denominator + message aggregation) run on the
TensorEngine with host-precomputed fp8 one-hot matrices (exact), accumulating
each target-block's edge tiles in PSUM.  The softmax division is commuted
past the segment sum and applied per node.  k-bias drops out exactly (softmax
shift invariance); v-bias is folded in after aggregation.  ACT runs only
copies + Exp (sigmoid via exp, layernorm rsqrt via DVE Newton iteration) so
the activation table is loaded once.
"""

import sys

sys.path.insert(0, "/opt/trn_rl_repo")

import numpy as np

N, IN_DIM, D, H, L, E, MAX_DEG = 50000, 128, 128, 16, 3, 800000, 512
C = D // H
P = 128
NCORES = 8
NSH = N // NCORES            # 6250 nodes per core
NBLK = (NSH + P - 1) // P    # 49 target blocks per core
NPAD = NBLK * P              # 6272 padded rows per core
NB_ROWS = NCORES * NPAD // 2  # 25088 rows per src bucket (fits int16)

RSQRT_MAGIC = 0x5F3759DF


def _bf16(a):
    import ml_dtypes

    return np.asarray(a, dtype=ml_dtypes.bfloat16)


def _fp8(a):
    import ml_dtypes

    return np.asarray(a, dtype=ml_dtypes.float8_e4m3)


def _wrap_idx16(idx, pad_to=None, fill=0):
    """int16 idx array -> [128, n/16] wrapped (j -> [j%16, j//16]) and
    replicated across the 8 gpsimd cores' 16-partition groups."""
    n = len(idx) if pad_to is None else pad_to
    assert n % 16 == 0
    a = np.full(n, fill, dtype=np.int16)
    a[: len(idx)] = idx.astype(np.int16)
    w = a.reshape(n // 16, 16).T  # [16, n/16]
    return np.tile(w, (8, 1))  # [128, n/16]


def _preprocess(x, edge_index):
    """Host-side integer/index preprocessing + per-core shard arrays."""
    src = np.asarray(edge_index[0], dtype=np.int64)
    tgt = np.asarray(edge_index[1], dtype=np.int64)

    # degrees (int) for the centrality-embedding gather
    idg = np.clip(np.bincount(tgt, minlength=N), 0, MAX_DEG)
    odg = np.clip(np.bincount(src, minlength=N), 0, MAX_DEG)

    # global row in the AllGather'ed kv table of node g
    kv_row = (src // NSH) * NPAD + (src % NSH)
    bucket = (kv_row >= NB_ROWS).astype(np.int64)
    src_loc = kv_row - bucket * NB_ROWS  # 0..25087, int16-safe

    # first pass: find the max (block,bucket) run length across all cores
    run_max = 0
    per_core = []
    for c in range(NCORES):
        m = (tgt // NSH) == c
        cs, ct, cb, csl = src[m], tgt[m] - c * NSH, bucket[m], src_loc[m]
        blk = ct // P
        cnt = np.bincount(blk * 2 + cb, minlength=NBLK * 2)
        run_max = max(run_max, cnt.max())
        per_core.append((cs, ct, cb, csl, blk))
    trun = int((run_max + P - 1) // P)  # tiles per (block,bucket) run
    nrun = trun * P
    KIB = trun * 16                     # idx bytes per partition per run
    EDB = KIB + 2 * trun * P            # + sg fp8 + st fp8 bytes

    ncalls = (trun + GBATCH - 1) // GBATCH
    cores = []
    for c in range(NCORES):
        cs, ct, cb, csl, blk = per_core[c]
        order = np.lexsort((ct, cb, blk))
        cs, ct, cb, csl, blk = (a[order] for a in (cs, ct, cb, csl, blk))

        edata = np.zeros((P, NBLK * 2 * EDB), dtype=np.uint8)
        gcnt = np.zeros((NBLK * 2, ncalls), dtype=np.int32)
        for k in range(NBLK):
            for b in range(2):
                sel = (cb == b) & (blk == k)
                n_e = int(sel.sum())
                for ci, i0 in enumerate(range(0, trun, GBATCH)):
                    i1 = min(i0 + GBATCH, trun)
                    gcnt[k * 2 + b, ci] = max(0, min(n_e - i0 * P, (i1 - i0) * P))
                ki = _wrap_idx16(csl[sel], pad_to=nrun, fill=-1)  # [128, trun*8]
                tl = ct[sel] - k * P  # 0..127 col within the block
                ee = np.arange(n_e)
                S = np.zeros((P, nrun), dtype=np.float32)   # [e%128, t*128+tl]
                S[ee % P, (ee // P) * P + tl] = 1.0
                ST = np.zeros((P, nrun), dtype=np.float32)  # [tl, e]
                ST[tl, ee] = 1.0
                off = (k * 2 + b) * EDB
                edata[:, off : off + KIB] = ki.view(np.uint8)
                edata[:, off + KIB : off + KIB + nrun] = _fp8(S).view(np.uint8)
                edata[:, off + KIB + nrun : off + EDB] = _fp8(ST).view(np.uint8)

        cores.append(
            dict(
                edata=edata,
                gcnt=np.broadcast_to(gcnt.reshape(1, -1), (P, NBLK * 2 * ncalls)).copy(),
                idg=_wrap_idx16(np.pad(idg[c * NSH : (c + 1) * NSH], (0, NPAD - NSH))),
                odg=_wrap_idx16(np.pad(odg[c * NSH : (c + 1) * NSH], (0, NPAD - NSH))),
                x=np.pad(
                    np.asarray(x[c * NSH : (c + 1) * NSH], dtype=np.float32),
                    ((0, NPAD - NSH), (0, 0)),
                ),
            )
        )
    return cores, trun, EDB


import os as _os

PROBE_NO_COLLECTIVE = bool(int(_os.environ.get("KB_NOCOLL", "0")))
ABL_NOEDGE = bool(int(_os.environ.get("KB_NOEDGE", "0")))   # skip edge interior

GBATCH = int(_os.environ.get("KB_GBATCH", "5"))   # kv gather tiles per call
EBATCH = int(_os.environ.get("KB_EBATCH", "3"))   # emb gather blocks per call
EPBUFS = int(_os.environ.get("KB_EPBUFS", "4"))   # edge pool depth


def _build(trun, EDB):
    from concourse import bass, mybir
    import concourse.tile as tile
    from concourse.bacc import Bacc
    from concourse.masks import make_identity

    dt = mybir.dt
    AX = mybir.AxisListType
    OP = mybir.AluOpType
    AF = mybir.ActivationFunctionType

    KIB = trun * 16
    nrun = trun * P

    nc = Bacc(None, target_bir_lowering=False, debug=False, num_devices=NCORES,
              num_swdge_queues=4)
    qctr = [0]

    def _nextq():
        qctr[0] = (qctr[0] + 1) % 4
        return qctr[0]

    # ---- parameters (per core) -------------------------------------------
    xin = nc.declare_dram_parameter("x", [NPAD, D], dt.float32, isOutput=False)
    emb_i = nc.declare_dram_parameter("emb_in", [MAX_DEG + 1, D], dt.float32, isOutput=False)
    emb_o = nc.declare_dram_parameter("emb_out", [MAX_DEG + 1, D], dt.float32, isOutput=False)
    idg_p = nc.declare_dram_parameter("idg", [P, NPAD // 16], dt.int16, isOutput=False)
    odg_p = nc.declare_dram_parameter("odg", [P, NPAD // 16], dt.int16, isOutput=False)
    win_p = nc.declare_dram_parameter("win", [D, D], dt.bfloat16, isOutput=False)
    bin_p = nc.declare_dram_parameter("bin", [P, D], dt.float32, isOutput=False)
    wcat_p = nc.declare_dram_parameter("wcat", [D, L * 4 * D], dt.bfloat16, isOutput=False)
    bcat_p = nc.declare_dram_parameter("bcat", [P, L * 2 * D], dt.float32, isOutput=False)
    bvp_p = nc.declare_dram_parameter("bvp", [P, L * D], dt.float32, isOutput=False)
    lnp_p = nc.declare_dram_parameter("lnp", [P, L * 2 * D], dt.float32, isOutput=False)
    fnp_p = nc.declare_dram_parameter("fnp", [P, 2 * D], dt.float32, isOutput=False)
    wb_p = nc.declare_dram_parameter("wbeta", [P, L * 2 * D], dt.float32, isOutput=False)
    ed_p = nc.declare_dram_parameter("edata", [P, NBLK * 2 * EDB], dt.uint8, isOutput=False)
    NC_G = (trun + GBATCH - 1) // GBATCH
    gc_p = nc.declare_dram_parameter("gcnt", [P, NBLK * 2 * NC_G], dt.int32, isOutput=False)
    out_p = nc.declare_dram_parameter("out", [NSH, D], dt.float32, isOutput=True)

    # ---- DRAM scratch -----------------------------------------------------
    kvb = nc.dram_tensor("kv_bounce", [NPAD, 2 * D], dt.bfloat16)
    kvf = nc.dram_tensor("kv_full", [NCORES * NPAD, 2 * D], dt.bfloat16, addr_space="Shared")

    with tile.TileContext(nc) as tc:
        with (
            tc.tile_pool(name="persist", bufs=1) as pp,
            tc.tile_pool(name="wtiles", bufs=1) as wp,
            tc.tile_pool(name="work", bufs=1) as kp,
            tc.tile_pool(name="small", bufs=3) as sp,
            tc.tile_pool(name="edge", bufs=EPBUFS) as ep,
            tc.tile_pool(name="psA", bufs=1, space="PSUM") as psA,
            tc.tile_pool(name="psB", bufs=1, space="PSUM") as psB,
            tc.tile_pool(name="psC", bufs=2, space="PSUM") as psC,
            tc.tile_pool(name="psQ", bufs=2, space="PSUM") as psQ,
        ):
            # persistent state
            h = pp.tile([P, NBLK, D], dt.float32, tag="h")
            xr = pp.tile([P, NBLK, D], dt.bfloat16, tag="xr")
            qsb = pp.tile([P, NBLK, D], dt.bfloat16, tag="qsb")

            ident = wp.tile([P, P], dt.bfloat16, tag="ident")
            make_identity(nc, ident[:])
            win = wp.tile([D, D], dt.bfloat16, tag="win")
            nc.sync.dma_start(win[:], win_p.ap())
            bin_t = wp.tile([P, D], dt.float32, tag="bin")
            nc.sync.dma_start(bin_t[:], bin_p.ap())
            wcat = wp.tile([D, L, 4 * D], dt.bfloat16, tag="wcat")
            nc.sync.dma_start(wcat[:], wcat_p.ap())
            bcat = wp.tile([P, L, 2 * D], dt.float32, tag="bcat")
            nc.sync.dma_start(bcat[:], bcat_p.ap())
            bvt = wp.tile([P, L, D], dt.float32, tag="bvt")
            nc.sync.dma_start(bvt[:], bvp_p.ap())
            lnp = wp.tile([P, L, 2 * D], dt.float32, tag="lnp")
            nc.sync.dma_start(lnp[:], lnp_p.ap())
            fnp = wp.tile([P, 2 * D], dt.float32, tag="fnp")
            nc.sync.dma_start(fnp[:], fnp_p.ap())
            wb = wp.tile([P, L, 2 * D], dt.float32, tag="wb")
            nc.sync.dma_start(wb[:], wb_p.ap())
            gct = wp.tile([P, NBLK * 2 * NC_G], dt.int32, tag="gct")
            nc.sync.dma_start(gct[:], gc_p.ap())
            gregs = [nc.gpsimd.alloc_register(f"gcnt_reg{i}") for i in range(8)]
            gregc = [0]

            def _rsqrt(rs, ve):
                """rs = 1/sqrt(ve) via bit-hack seed + 2 Newton iterations.
                rs, ve: [P, 1] f32 tiles (DVE only — no ACT table)."""
                iv = sp.tile([P, 1], dt.int32, tag="nw_i")
                nc.vector.tensor_scalar(
                    out=iv[:], in0=ve[:].bitcast(dt.int32), scalar1=1,
                    scalar2=None, op0=OP.logical_shift_right,
                )
                nc.vector.tensor_scalar(
                    out=iv[:], in0=iv[:], scalar1=-1, scalar2=RSQRT_MAGIC,
                    op0=OP.mult, op1=OP.add,
                )
                y = iv[:].bitcast(dt.float32)
                t = sp.tile([P, 1], dt.float32, tag="nw_t")
                for _ in range(2):
                    nc.vector.tensor_tensor(out=t[:], in0=y, in1=y, op=OP.mult)
                    nc.vector.tensor_tensor(out=t[:], in0=t[:], in1=ve[:], op=OP.mult)
                    nc.vector.tensor_scalar(
                        out=t[:], in0=t[:], scalar1=-0.5, scalar2=1.5,
                        op0=OP.mult, op1=OP.add,
                    )
                    nc.vector.tensor_tensor(out=t[:], in0=y, in1=t[:], op=OP.mult)
                    nc.vector.tensor_copy(iv[:].bitcast(dt.float32), t[:])
                nc.vector.tensor_copy(rs[:], y)

            def _ln_to(hb, t, scale_ap, bias_ap):
                """hb[P, D] (bf16) = LN(h[:, t, :]) * scale + bias."""
                stats = sp.tile([P, 6], dt.float32, tag="bst")
                nc.vector.bn_stats(stats[:], h[:, t, :])
                mv = sp.tile([P, 2], dt.float32, tag="mv")
                nc.vector.bn_aggr(mv[:], stats[:])
                ve = sp.tile([P, 1], dt.float32, tag="ve")
                nc.vector.tensor_scalar_add(ve[:], mv[:, 1:2], 1e-5)
                rs = sp.tile([P, 1], dt.float32, tag="rs")
                _rsqrt(rs, ve)
                hf = sp.tile([P, D], dt.float32, tag="hf")
                nc.vector.tensor_tensor(
                    out=hf[:], in0=h[:, t, :],
                    in1=mv[:, 0:1].to_broadcast([P, D]), op=OP.subtract,
                )
                nc.vector.scalar_tensor_tensor(
                    out=hb[:], in0=hf[:], scalar=rs[:], in1=scale_ap,
                    op0=OP.mult, op1=OP.mult,
                )
                nc.vector.tensor_tensor(out=hb[:], in0=hb[:], in1=bias_ap, op=OP.add)

            def _lnproj_block(t, layer):
                """LN h[:,t] (lnp[layer]) -> proj (wcat[layer]) -> kvb/qsb/xr."""
                hb = sp.tile([P, D], dt.bfloat16, tag="hb")
                _ln_to(hb, t, lnp[:, layer, 0:D], lnp[:, layer, D : 2 * D])
                pT = psA.tile([P, P], dt.bfloat16, tag="pT")
                nc.tensor.transpose(out=pT[:], in_=hb[:], identity=ident[:])
                hnTt = sp.tile([P, D], dt.bfloat16, tag="hnTt")
                nc.scalar.copy(hnTt[:], pT[:])
                ps = psB.tile([P, 4 * D], dt.float32, tag="ps")
                nc.tensor.matmul(
                    out=ps[:], lhsT=hnTt[:], rhs=wcat[:, layer, :],
                    start=True, stop=True,
                )
                kvq = sp.tile([P, 2 * D], dt.bfloat16, tag="kvq")
                nc.scalar.copy(kvq[:], ps[:, 0 : 2 * D])
                nc.vector.scalar_tensor_tensor(
                    out=qsb[:, t, :], in0=ps[:, 2 * D : 3 * D], scalar=1.0,
                    in1=bcat[:, layer, 0:D], op0=OP.mult, op1=OP.add,
                )
                nc.vector.scalar_tensor_tensor(
                    out=xr[:, t, :], in0=ps[:, 3 * D : 4 * D], scalar=1.0,
                    in1=bcat[:, layer, D : 2 * D], op0=OP.mult, op1=OP.add,
                )
                nc.sync.dma_start(kvb.ap()[t * P : (t + 1) * P, :], kvq[:])

            def _final_block(t):
                """Final LN on h[:,t] -> out DMA."""
                ot = sp.tile([P, D], dt.float32, tag="ot")
                stats = sp.tile([P, 6], dt.float32, tag="bst")
                nc.vector.bn_stats(stats[:], h[:, t, :])
                mv = sp.tile([P, 2], dt.float32, tag="mv")
                nc.vector.bn_aggr(mv[:], stats[:])
                ve = sp.tile([P, 1], dt.float32, tag="ve")
                nc.vector.tensor_scalar_add(ve[:], mv[:, 1:2], 1e-5)
                rs = sp.tile([P, 1], dt.float32, tag="rs")
                _rsqrt(rs, ve)
                nc.vector.tensor_tensor(
                    out=ot[:], in0=h[:, t, :],
                    in1=mv[:, 0:1].to_broadcast([P, D]), op=OP.subtract,
                )
                nc.vector.scalar_tensor_tensor(
                    out=ot[:], in0=ot[:], scalar=rs[:], in1=fnp[:, 0:D],
                    op0=OP.mult, op1=OP.mult,
                )
                nc.vector.tensor_tensor(
                    out=ot[:], in0=ot[:], in1=fnp[:, D : 2 * D], op=OP.add
                )
                lo = t * P
                hi = min((t + 1) * P, NSH)
                if hi > lo:
                    nc.sync.dma_start(out_p.ap()[lo:hi, :], ot[0 : hi - lo, :])

            # ---- phase 0: h = x @ W_in + b_in + emb_in[idg] + emb_out[odg]
            for t in range(NBLK):
                xt = sp.tile([P, D], dt.float32, tag="xt")
                nc.sync.dma_start(xt[:], xin.ap()[t * P : (t + 1) * P, :])
                xb = sp.tile([P, D], dt.bfloat16, tag="xb")
                nc.vector.tensor_copy(xb[:], xt[:])
                pT = psA.tile([P, P], dt.bfloat16, tag="pT")
                nc.tensor.transpose(out=pT[:], in_=xb[:], identity=ident[:])
                xTb = sp.tile([P, D], dt.bfloat16, tag="xTb")
                nc.scalar.copy(xTb[:], pT[:])
                ph = psB.tile([P, D], dt.float32, tag="ph")
                nc.tensor.matmul(out=ph[:], lhsT=xTb[:], rhs=win[:], start=True, stop=True)
                nc.vector.scalar_tensor_tensor(
                    out=h[:, t, :], in0=ph[:], scalar=1.0, in1=bin_t[:],
                    op0=OP.mult, op1=OP.add,
                )
            for tabl, idxp in ((emb_i, idg_p), (emb_o, odg_p)):
                gi = kp.tile([P, NPAD // 16], dt.int16, tag="gidx")
                nc.sync.dma_start(gi[:], idxp.ap())
                eg = kp.tile([P, NBLK, D], dt.float32, tag="scratch")
                for i0 in range(0, NBLK, EBATCH):
                    i1 = min(i0 + EBATCH, NBLK)
                    nidx = (i1 - i0) * P
                    nc.gpsimd.dma_gather(
                        out_ap=eg[:, i0:i1, :], in_ap=tabl.ap(),
                        idxs_ap=gi[:, i0 * 8 : i1 * 8],
                        num_idxs=nidx, num_idxs_reg=nidx, elem_size=D,
                        queue_num=_nextq(),
                    )
                nc.vector.tensor_tensor(out=h[:], in0=h[:], in1=eg[:], op=OP.add)

            # zero the kvg pool buffers once (trimmed gathers leave stale
            # bytes behind; first use must not see NaN bit patterns)
            for _ in range(EPBUFS):
                z = ep.tile([P, trun, 2 * D], dt.bfloat16, tag="kvg")
                nc.vector.memset(z[:], 0.0)

            # ---- pre-pass: LN + projections for layer 0
            for t in range(NBLK):
                _lnproj_block(t, 0)

            # ---- layers ----------------------------------------------------
            for layer in range(L):
                if PROBE_NO_COLLECTIVE:
                    nc.gpsimd.dma_start(out=kvf.ap()[0:NPAD, :], in_=kvb.ap())
                else:
                    nc.gpsimd.collective_compute(
                        "AllGather",
                        OP.bypass,
                        replica_groups=[list(range(NCORES))],
                        ins=[kvb.ap().opt()],
                        outs=[kvf.ap().opt()],
                    )

                # ---- edge phase: per (tgt block, bucket) run of trun tiles
                for blk in range(NBLK):
                    pm = psC.tile([P, D + H], dt.float32, tag="pm")
                    for b in range(2):
                        off = (blk * 2 + b) * EDB
                        ed = ep.tile([P, EDB], dt.uint8, tag="ed")
                        nc.sync.dma_start(ed[:], ed_p.ap()[:, off : off + EDB])
                        ki = ed[:, 0:KIB].bitcast(dt.int16)         # [P, trun*8]
                        sgv = ed[:, KIB : KIB + nrun].bitcast(dt.float8e4).rearrange(
                            "p (t e) -> p t e", e=P
                        )
                        stv = ed[:, KIB + nrun : EDB].bitcast(dt.float8e4).rearrange(
                            "p (t e) -> p t e", e=P
                        )

                        if ABL_NOEDGE:
                            ue0 = ep.tile([P, trun, D + H], dt.bfloat16, tag="ue")
                            nc.vector.memset(ue0[:], 0.5)
                            for tt in range(trun):
                                nc.tensor.matmul(
                                    out=pm[:], lhsT=sgv[:, tt, :], rhs=ue0[:, tt, :],
                                    start=(b == 0 and tt == 0),
                                    stop=(b == 1 and tt == trun - 1),
                                )
                            continue
                        kvg = ep.tile([P, trun, 2 * D], dt.bfloat16, tag="kvg")
                        for ci, i0 in enumerate(range(0, trun, GBATCH)):
                            i1 = min(i0 + GBATCH, trun)
                            nidx = (i1 - i0) * P
                            gj = (blk * 2 + b) * NC_G + ci
                            greg = gregs[gregc[0] % len(gregs)]
                            gregc[0] += 1
                            nc.gpsimd.reg_load(greg, gct[0:1, gj : gj + 1])
                            nc.gpsimd.dma_gather(
                                out_ap=kvg[:, i0:i1, :],
                                in_ap=kvf.ap()[b * NB_ROWS : (b + 1) * NB_ROWS, :],
                                idxs_ap=ki[:, i0 * 8 : i1 * 8],
                                num_idxs=nidx, num_idxs_reg=greg,
                                elem_size=2 * D,
                                queue_num=_nextq(),
                            )
                        # q-broadcast via PE: qg[e, f] = q[tl(e), f]
                        qg = ep.tile([P, trun, D], dt.bfloat16, tag="qg")
                        for c0 in range(0, nrun, 512):
                            c1 = min(c0 + 512, nrun)
                            qp = psQ.tile([P, 512], dt.float32, tag="qp")
                            for tt in range(c0 // P, c1 // P):
                                o = tt * P - c0
                                nc.tensor.matmul(
                                    out=qp[:, o : o + P], lhsT=stv[:, tt, :],
                                    rhs=qsb[:, blk, :], start=True, stop=True,
                                )
                            nc.scalar.copy(
                                qg[:, c0 // P : c1 // P, :],
                                qp[:, 0 : c1 - c0].rearrange("p (t e) -> p t e", e=P),
                            )
                        # per-edge logits: alpha = sum_c q*k (tree reduce)
                        qk = ep.tile([P, trun, H, C], dt.bfloat16, tag="qk")
                        nc.vector.tensor_tensor(
                            out=qk[:].rearrange("p t h c -> p t (h c)"),
                            in0=qg[:], in1=kvg[:, :, 0:D], op=OP.mult,
                        )
                        t1 = ep.tile([P, trun, H, 4], dt.bfloat16, tag="t1")
                        with nc.allow_low_precision(reason="alpha logits are O(0.1)"):
                            nc.vector.tensor_tensor(
                                out=t1[:], in0=qk[:, :, :, 0:4], in1=qk[:, :, :, 4:8],
                                op=OP.add,
                            )
                            t2 = ep.tile([P, trun, H, 2], dt.bfloat16, tag="t2")
                            nc.vector.tensor_tensor(
                                out=t2[:], in0=t1[:, :, :, 0:2], in1=t1[:, :, :, 2:4],
                                op=OP.add,
                            )
                            al = ep.tile([P, trun, H, 1], dt.bfloat16, tag="al")
                            nc.vector.tensor_tensor(
                                out=al[:], in0=t2[:, :, :, 0:1], in1=t2[:, :, :, 1:2],
                                op=OP.add,
                            )
                        ue = ep.tile([P, trun, D + H], dt.bfloat16, tag="ue")
                        nc.scalar.activation(
                            out=ue[:, :, D : D + H].rearrange("p t (h o) -> p t h o", o=1),
                            in_=al[:], func=AF.Exp,
                        )
                        wex = ep.tile([P, trun, H, C], dt.bfloat16, tag="wex")
                        nc.scalar.activation(
                            out=wex[:], in_=al[:].to_broadcast([P, trun, H, C]),
                            func=AF.Exp,
                        )
                        nc.vector.tensor_tensor(
                            out=ue[:, :, 0:D], in0=kvg[:, :, D : 2 * D],
                            in1=wex[:].rearrange("p t h c -> p t (h c)"), op=OP.mult,
                        )
                        for tt in range(trun):
                            nc.tensor.matmul(
                                out=pm[:], lhsT=sgv[:, tt, :], rhs=ue[:, tt, :],
                                start=(b == 0 and tt == 0),
                                stop=(b == 1 and tt == trun - 1),
                            )

                    # ---- fused per-block tail: normalize, gate, residual,
                    # then next layer's LN+projection (or final LN) ----------
                    msgb = sp.tile([P, D], dt.float32, tag="msgb")
                    rden = sp.tile([P, H, 1], dt.float32, tag="rden")
                    nc.vector.tensor_scalar_add(
                        rden[:], pm[:, D : D + H].rearrange("p (h o) -> p h o", o=1), 1e-20
                    )
                    nc.vector.reciprocal(rden[:], rden[:])
                    nc.vector.tensor_tensor(
                        out=msgb[:].rearrange("p (h c) -> p h c", c=C),
                        in0=pm[:, 0:D].rearrange("p (h c) -> p h c", c=C),
                        in1=rden[:].to_broadcast([P, H, C]),
                        op=OP.mult,
                    )
                    nc.vector.tensor_tensor(
                        out=msgb[:], in0=msgb[:], in1=bvt[:, layer, :], op=OP.add
                    )
                    scr = sp.tile([P, D], dt.float32, tag="scr")
                    bs1 = sp.tile([P, 1], dt.float32, tag="bs1")
                    nc.vector.scalar_tensor_tensor(
                        out=scr[:], in0=msgb[:], scalar=1.0, in1=wb[:, layer, 0:D],
                        op0=OP.mult, op1=OP.mult, accum_out=bs1[:],
                    )
                    bs2 = sp.tile([P, 1], dt.float32, tag="bs2")
                    nc.vector.scalar_tensor_tensor(
                        out=scr[:], in0=xr[:, blk, :], scalar=1.0, in1=wb[:, layer, D : 2 * D],
                        op0=OP.mult, op1=OP.mult, accum_out=bs2[:],
                    )
                    nc.vector.tensor_tensor(out=bs1[:], in0=bs1[:], in1=bs2[:], op=OP.add)
                    beta = sp.tile([P, 1], dt.float32, tag="beta")
                    nc.scalar.activation(out=beta[:], in_=bs1[:], func=AF.Exp, scale=-1.0)
                    nc.vector.tensor_scalar_add(beta[:], beta[:], 1.0)
                    nc.vector.reciprocal(beta[:], beta[:])
                    # h += msg + beta*(xr - msg)
                    tmpb = sp.tile([P, D], dt.float32, tag="tmpb")
                    nc.vector.tensor_tensor(
                        out=tmpb[:], in0=xr[:, blk, :], in1=msgb[:], op=OP.subtract
                    )
                    nc.vector.scalar_tensor_tensor(
                        out=tmpb[:], in0=tmpb[:], scalar=beta[:], in1=msgb[:],
                        op0=OP.mult, op1=OP.add,
                    )
                    nc.vector.tensor_tensor(
                        out=h[:, blk, :], in0=h[:, blk, :], in1=tmpb[:], op=OP.add
                    )
                    if layer == L - 1:
                        _final_block(blk)
                    else:
                        _lnproj_block(blk, layer + 1)

    nc.finalize()
    return nc

LAST_RES = None


def _make_in_maps(inputs, cores):
    sq = 1.0 / np.sqrt(np.float32(C))
    Wq, Wk, Wv, Wsk = (np.asarray(inputs[k], dtype=np.float32) for k in ("Wq", "Wk", "Wv", "Wskip"))
    bq, bv, bsk = (np.asarray(inputs[k], dtype=np.float32) for k in ("bq", "bv", "bskip"))
    # order per layer: k | v | q*sq | skip  (k-bias dropped: softmax shift
    # invariance; v-bias folded in post-aggregation)
    wcat = np.concatenate([Wk, Wv, Wq * sq, Wsk], axis=2).transpose(1, 0, 2).reshape(D, L * 4 * D)
    bcat = np.concatenate([bq * sq, bsk], axis=1)  # [L, 2D]
    bcat_rep = np.broadcast_to(bcat[:, None, :], (L, P, 2 * D)).transpose(1, 0, 2).reshape(P, L * 2 * D).copy()
    bvp = np.broadcast_to(bv[:, None, :], (L, P, D)).transpose(1, 0, 2).reshape(P, L * D).copy()
    lns, lnb = np.asarray(inputs["ln_scale"], np.float32), np.asarray(inputs["ln_bias"], np.float32)
    lnp = np.broadcast_to(
        np.concatenate([lns, lnb], axis=1)[:, None, :], (L, P, 2 * D)
    ).transpose(1, 0, 2).reshape(P, L * 2 * D).copy()
    fnp = np.broadcast_to(
        np.concatenate([inputs["fn_scale"], inputs["fn_bias"]])[None, :], (P, 2 * D)
    ).astype(np.float32).copy()
    Wbeta = np.asarray(inputs["Wbeta"], np.float32)  # [L, 3D, 1]
    wa = Wbeta[:, 0:D, 0] + Wbeta[:, 2 * D : 3 * D, 0]      # msg coeff
    wbx = Wbeta[:, D : 2 * D, 0] - Wbeta[:, 2 * D : 3 * D, 0]  # xr coeff
    wbeta_rep = np.broadcast_to(
        np.concatenate([wa, wbx], axis=1)[:, None, :], (L, P, 2 * D)
    ).transpose(1, 0, 2).reshape(P, L * 2 * D).copy()
    bin_rep = np.broadcast_to(
        np.asarray(inputs["b_in"], np.float32)[None, :], (P, D)
    ).copy()

    common = dict(
        emb_in=np.asarray(inputs["in_emb"], np.float32),
        emb_out=np.asarray(inputs["out_emb"], np.float32),
        win=_bf16(inputs["W_in"]),
        bin=bin_rep,
        wcat=_bf16(wcat),
        bcat=bcat_rep,
        bvp=bvp,
        lnp=lnp,
        fnp=fnp,
        wbeta=wbeta_rep,
    )
    in_maps = []
    for c in range(NCORES):
        m = dict(common)
        cd = cores[c]
        m.update(x=cd["x"], idg=cd["idg"], odg=cd["odg"], edata=cd["edata"],
                 gcnt=cd["gcnt"])
        in_maps.append(m)
    return in_maps


def kernel(**inputs):
    import os

    from concourse.bass_utils import run_bass_kernel_spmd

    x = np.asarray(inputs["x"], dtype=np.float32)
    edge_index = np.asarray(inputs["edge_index"])
    cores, trun, EDB = _preprocess(x, edge_index)
    in_maps = _make_in_maps(inputs, cores)

    nc = _build(trun, EDB)
    kw = {}
    td = os.environ.get("BASS_KERNEL_TMPDIR")
    if td:
        kw["tmpdir"] = td
    res = run_bass_kernel_spmd(nc, in_maps, core_ids=list(range(NCORES)), **kw)
    global LAST_RES
    LAST_RES = res
    outs = [np.asarray(r["out"], dtype=np.float32) for r in res.results]
    return np.concatenate(outs, axis=0)


if __name__ == "__main__":
    import reference

    inp = {k: np.asarray(v) for k, v in reference.setup_inputs().items()}
    exp = np.asarray(reference.reference(**inp))
    act = kernel(**inp)
    err = np.abs(act - exp).max() / (np.abs(exp).max() + 1e-9)
    print("Relative error:", err)


# revision 22
# speedup vs baseline: 2.4738x; 1.3543x over previous
"""Graphormer-expert GNN kernel for 8 Trainium2 NeuronCores.

Strategy (matches the sharding hint): nodes are partitioned 8 x 6250 (graph
parallel); each core owns the edges whose *target* falls in its shard, so the
scatter-softmax is core-local.  Per layer each core computes LN + projections
for its own nodes, k|v rows (bf16) are exchanged with an AllGather, and
per-edge k/v rows are fetched with SWDGE dma_gather (int16 indices; source
table split in two 25088-row buckets; padding indices are -1 so the Q7
descriptor generator trims them).  Per-edge softmax runs without
max-subtraction (|alpha| << 1 for this model); the q-row broadcast and the
segment sums (softmax denominator + message aggregation) run on the
TensorEngine with host-precomputed fp8 one-hot matrices (exact), accumulating
each target-block's edge tiles in PSUM.  The softmax division is commuted
past the segment sum and applied per node.  k-bias drops out exactly (softmax
shift invariance); v-bias is folded in after aggregation.  ACT runs only
copies + Exp (sigmoid via exp, layernorm rsqrt via DVE Newton iteration) so
the activation table is loaded once.
"""

import sys

sys.path.insert(0, "/opt/trn_rl_repo")

import numpy as np

N, IN_DIM, D, H, L, E, MAX_DEG = 50000, 128, 128, 16, 3, 800000, 512
C = D // H
P = 128
NCORES = 8
NSH = N // NCORES            # 6250 nodes per core
NBLK = (NSH + P - 1) // P    # 49 target blocks per core
NPAD = NBLK * P              # 6272 padded rows per core
NB_ROWS = NCORES * NPAD // 2  # 25088 rows per src bucket (fits int16)

RSQRT_MAGIC = 0x5F3759DF


def _bf16(a):
    import ml_dtypes

    return np.asarray(a, dtype=ml_dtypes.bfloat16)


def _fp8(a):
    import ml_dtypes

    return np.asarray(a, dtype=ml_dtypes.float8_e4m3)


def _wrap_idx16(idx, pad_to=None, fill=0):
    """int16 idx array -> [128, n/16] wrapped (j -> [j%16, j//16]) and
    replicated across the 8 gpsimd cores' 16-partition groups."""
    n = len(idx) if pad_to is None else pad_to
    assert n % 16 == 0
    a = np.full(n, fill, dtype=np.int16)
    a[: len(idx)] = idx.astype(np.int16)
    w = a.reshape(n // 16, 16).T  # [16, n/16]
    return np.tile(w, (8, 1))  # [128, n/16]


def _preprocess(x, edge_index):
    """Host-side integer/index preprocessing + per-core shard arrays."""
    src = np.asarray(edge_index[0], dtype=np.int64)
    tgt = np.asarray(edge_index[1], dtype=np.int64)

    # degrees (int) for the centrality-embedding gather
    idg = np.clip(np.bincount(tgt, minlength=N), 0, MAX_DEG)
    odg = np.clip(np.bincount(src, minlength=N), 0, MAX_DEG)

    # global row in the AllGather'ed kv table of node g
    kv_row = (src // NSH) * NPAD + (src % NSH)
    bucket = (kv_row >= NB_ROWS).astype(np.int64)
    src_loc = kv_row - bucket * NB_ROWS  # 0..25087, int16-safe

    # first pass: find the max (block,bucket) run length across all cores
    run_max = 0
    per_core = []
    for c in range(NCORES):
        m = (tgt // NSH) == c
        cs, ct, cb, csl = src[m], tgt[m] - c * NSH, bucket[m], src_loc[m]
        blk = ct // P
        cnt = np.bincount(blk * 2 + cb, minlength=NBLK * 2)
        run_max = max(run_max, cnt.max())
        per_core.append((cs, ct, cb, csl, blk))
    trun = int((run_max + P - 1) // P)  # tiles per (block,bucket) run
    nrun = trun * P
    KIB = trun * 16                     # idx bytes per partition per run
    EDB = KIB + 2 * trun * P            # + sg fp8 + st fp8 bytes

    ncalls = (trun + GBATCH - 1) // GBATCH
    cores = []
    for c in range(NCORES):
        cs, ct, cb, csl, blk = per_core[c]
        order = np.lexsort((ct, cb, blk))
        cs, ct, cb, csl, blk = (a[order] for a in (cs, ct, cb, csl, blk))

        edata = np.zeros((P, NBLK * 2 * EDB), dtype=np.uint8)
        gcnt = np.zeros((NBLK * 2, ncalls), dtype=np.int32)
        for k in range(NBLK):
            for b in range(2):
                sel = (cb == b) & (blk == k)
                n_e = int(sel.sum())
                for ci, i0 in enumerate(range(0, trun, GBATCH)):
                    i1 = min(i0 + GBATCH, trun)
                    gcnt[k * 2 + b, ci] = max(0, min(n_e - i0 * P, (i1 - i0) * P))
                ki = _wrap_idx16(csl[sel], pad_to=nrun, fill=-1)  # [128, trun*8]
                tl = ct[sel] - k * P  # 0..127 col within the block
                ee = np.arange(n_e)
                S = np.zeros((P, nrun), dtype=np.float32)   # [e%128, t*128+tl]
                S[ee % P, (ee // P) * P + tl] = 1.0
                ST = np.zeros((P, nrun), dtype=np.float32)  # [tl, e]
                ST[tl, ee] = 1.0
                off = (k * 2 + b) * EDB
                edata[:, off : off + KIB] = ki.view(np.uint8)
                edata[:, off + KIB : off + KIB + nrun] = _fp8(S).view(np.uint8)
                edata[:, off + KIB + nrun : off + EDB] = _fp8(ST).view(np.uint8)

        cores.append(
            dict(
                edata=edata,
                gcnt=np.broadcast_to(gcnt.reshape(1, -1), (P, NBLK * 2 * ncalls)).copy(),
                idg=_wrap_idx16(np.pad(idg[c * NSH : (c + 1) * NSH], (0, NPAD - NSH))),
                odg=_wrap_idx16(np.pad(odg[c * NSH : (c + 1) * NSH], (0, NPAD - NSH))),
                x=np.pad(
                    np.asarray(x[c * NSH : (c + 1) * NSH], dtype=np.float32),
                    ((0, NPAD - NSH), (0, 0)),
                ),
            )
        )
    return cores, trun, EDB


import os as _os

PROBE_NO_COLLECTIVE = bool(int(_os.environ.get("KB_NOCOLL", "0")))
ABL_NOEDGE = bool(int(_os.environ.get("KB_NOEDGE", "0")))   # skip edge interior

GBATCH = int(_os.environ.get("KB_GBATCH", "5"))   # kv gather tiles per call
EBATCH = int(_os.environ.get("KB_EBATCH", "3"))   # emb gather blocks per call
EPBUFS = int(_os.environ.get("KB_EPBUFS", "4"))   # edge pool depth
GPBUFS = int(_os.environ.get("KB_GPBUFS", "8"))   # gather pool depth (ed+kvg)


def _build(trun, EDB):
    from concourse import bass, mybir
    import concourse.tile as tile
    from concourse.bacc import Bacc
    from concourse.masks import make_identity

    dt = mybir.dt
    AX = mybir.AxisListType
    OP = mybir.AluOpType
    AF = mybir.ActivationFunctionType

    KIB = trun * 16
    nrun = trun * P

    nc = Bacc(None, target_bir_lowering=False, debug=False, num_devices=NCORES,
              num_swdge_queues=4)
    qctr = [0]

    def _nextq():
        qctr[0] = (qctr[0] + 1) % 4
        return qctr[0]

    # ---- parameters (per core) -------------------------------------------
    xin = nc.declare_dram_parameter("x", [NPAD, D], dt.float32, isOutput=False)
    emb_i = nc.declare_dram_parameter("emb_in", [MAX_DEG + 1, D], dt.float32, isOutput=False)
    emb_o = nc.declare_dram_parameter("emb_out", [MAX_DEG + 1, D], dt.float32, isOutput=False)
    idg_p = nc.declare_dram_parameter("idg", [P, NPAD // 16], dt.int16, isOutput=False)
    odg_p = nc.declare_dram_parameter("odg", [P, NPAD // 16], dt.int16, isOutput=False)
    win_p = nc.declare_dram_parameter("win", [D, D], dt.bfloat16, isOutput=False)
    bin_p = nc.declare_dram_parameter("bin", [P, D], dt.float32, isOutput=False)
    wcat_p = nc.declare_dram_parameter("wcat", [D, L * 4 * D], dt.bfloat16, isOutput=False)
    bcat_p = nc.declare_dram_parameter("bcat", [P, L * 2 * D], dt.float32, isOutput=False)
    bvp_p = nc.declare_dram_parameter("bvp", [P, L * D], dt.float32, isOutput=False)
    lnp_p = nc.declare_dram_parameter("lnp", [P, L * 2 * D], dt.float32, isOutput=False)
    fnp_p = nc.declare_dram_parameter("fnp", [P, 2 * D], dt.float32, isOutput=False)
    wb_p = nc.declare_dram_parameter("wbeta", [P, L * 2 * D], dt.float32, isOutput=False)
    ed_p = nc.declare_dram_parameter("edata", [P, NBLK * 2 * EDB], dt.uint8, isOutput=False)
    NC_G = (trun + GBATCH - 1) // GBATCH
    gc_p = nc.declare_dram_parameter("gcnt", [P, NBLK * 2 * NC_G], dt.int32, isOutput=False)
    out_p = nc.declare_dram_parameter("out", [NSH, D], dt.float32, isOutput=True)

    # ---- DRAM scratch -----------------------------------------------------
    kvb = nc.dram_tensor("kv_bounce", [NPAD, 2 * D], dt.bfloat16)
    kvf = nc.dram_tensor("kv_full", [NCORES * NPAD, 2 * D], dt.bfloat16, addr_space="Shared")

    with tile.TileContext(nc) as tc:
        with (
            tc.tile_pool(name="persist", bufs=1) as pp,
            tc.tile_pool(name="wtiles", bufs=1) as wp,
            tc.tile_pool(name="work", bufs=1) as kp,
            tc.tile_pool(name="small", bufs=3) as sp,
            tc.tile_pool(name="edge", bufs=EPBUFS) as ep,
            tc.tile_pool(name="gath", bufs=GPBUFS) as gp,
            tc.tile_pool(name="psA", bufs=1, space="PSUM") as psA,
            tc.tile_pool(name="psB", bufs=1, space="PSUM") as psB,
            tc.tile_pool(name="psC", bufs=2, space="PSUM") as psC,
            tc.tile_pool(name="psQ", bufs=2, space="PSUM") as psQ,
        ):
            # persistent state
            h = pp.tile([P, NBLK, D], dt.float32, tag="h")
            xr = pp.tile([P, NBLK, D], dt.bfloat16, tag="xr")
            qsb = pp.tile([P, NBLK, D], dt.bfloat16, tag="qsb")

            ident = wp.tile([P, P], dt.bfloat16, tag="ident")
            make_identity(nc, ident[:])
            win = wp.tile([D, D], dt.bfloat16, tag="win")
            nc.sync.dma_start(win[:], win_p.ap())
            bin_t = wp.tile([P, D], dt.float32, tag="bin")
            nc.sync.dma_start(bin_t[:], bin_p.ap())
            wcat = wp.tile([D, L, 4 * D], dt.bfloat16, tag="wcat")
            nc.sync.dma_start(wcat[:], wcat_p.ap())
            bcat = wp.tile([P, L, 2 * D], dt.float32, tag="bcat")
            nc.sync.dma_start(bcat[:], bcat_p.ap())
            bvt = wp.tile([P, L, D], dt.float32, tag="bvt")
            nc.sync.dma_start(bvt[:], bvp_p.ap())
            lnp = wp.tile([P, L, 2 * D], dt.float32, tag="lnp")
            nc.sync.dma_start(lnp[:], lnp_p.ap())
            fnp = wp.tile([P, 2 * D], dt.float32, tag="fnp")
            nc.sync.dma_start(fnp[:], fnp_p.ap())
            wb = wp.tile([P, L, 2 * D], dt.float32, tag="wb")
            nc.sync.dma_start(wb[:], wb_p.ap())
            gct = wp.tile([P, NBLK * 2 * NC_G], dt.int32, tag="gct")
            nc.sync.dma_start(gct[:], gc_p.ap())
            gregs = [nc.gpsimd.alloc_register(f"gcnt_reg{i}") for i in range(8)]
            gregc = [0]

            def _rsqrt(rs, ve):
                """rs = 1/sqrt(ve) via bit-hack seed + 2 Newton iterations.
                rs, ve: [P, 1] f32 tiles (DVE only — no ACT table)."""
                iv = sp.tile([P, 1], dt.int32, tag="nw_i")
                nc.vector.tensor_scalar(
                    out=iv[:], in0=ve[:].bitcast(dt.int32), scalar1=1,
                    scalar2=None, op0=OP.logical_shift_right,
                )
                nc.vector.tensor_scalar(
                    out=iv[:], in0=iv[:], scalar1=-1, scalar2=RSQRT_MAGIC,
                    op0=OP.mult, op1=OP.add,
                )
                y = iv[:].bitcast(dt.float32)
                t = sp.tile([P, 1], dt.float32, tag="nw_t")
                for _ in range(2):
                    nc.vector.tensor_tensor(out=t[:], in0=y, in1=y, op=OP.mult)
                    nc.vector.tensor_tensor(out=t[:], in0=t[:], in1=ve[:], op=OP.mult)
                    nc.vector.tensor_scalar(
                        out=t[:], in0=t[:], scalar1=-0.5, scalar2=1.5,
                        op0=OP.mult, op1=OP.add,
                    )
                    nc.vector.tensor_tensor(out=t[:], in0=y, in1=t[:], op=OP.mult)
                    nc.vector.tensor_copy(iv[:].bitcast(dt.float32), t[:])
                nc.vector.tensor_copy(rs[:], y)

            def _ln_to(hb, t, scale_ap, bias_ap):
                """hb[P, D] (bf16) = LN(h[:, t, :]) * scale + bias."""
                stats = sp.tile([P, 6], dt.float32, tag="bst")
                nc.vector.bn_stats(stats[:], h[:, t, :])
                mv = sp.tile([P, 2], dt.float32, tag="mv")
                nc.vector.bn_aggr(mv[:], stats[:])
                ve = sp.tile([P, 1], dt.float32, tag="ve")
                nc.vector.tensor_scalar_add(ve[:], mv[:, 1:2], 1e-5)
                rs = sp.tile([P, 1], dt.float32, tag="rs")
                _rsqrt(rs, ve)
                hf = sp.tile([P, D], dt.float32, tag="hf")
                nc.vector.tensor_tensor(
                    out=hf[:], in0=h[:, t, :],
                    in1=mv[:, 0:1].to_broadcast([P, D]), op=OP.subtract,
                )
                nc.vector.scalar_tensor_tensor(
                    out=hb[:], in0=hf[:], scalar=rs[:], in1=scale_ap,
                    op0=OP.mult, op1=OP.mult,
                )
                nc.vector.tensor_tensor(out=hb[:], in0=hb[:], in1=bias_ap, op=OP.add)

            def _lnproj_block(t, layer):
                """LN h[:,t] (lnp[layer]) -> proj (wcat[layer]) -> kvb/qsb/xr."""
                hb = sp.tile([P, D], dt.bfloat16, tag="hb")
                _ln_to(hb, t, lnp[:, layer, 0:D], lnp[:, layer, D : 2 * D])
                pT = psA.tile([P, P], dt.bfloat16, tag="pT")
                nc.tensor.transpose(out=pT[:], in_=hb[:], identity=ident[:])
                hnTt = sp.tile([P, D], dt.bfloat16, tag="hnTt")
                nc.scalar.copy(hnTt[:], pT[:])
                ps = psB.tile([P, 4 * D], dt.float32, tag="ps")
                nc.tensor.matmul(
                    out=ps[:], lhsT=hnTt[:], rhs=wcat[:, layer, :],
                    start=True, stop=True,
                )
                kvq = sp.tile([P, 2 * D], dt.bfloat16, tag="kvq")
                nc.scalar.copy(kvq[:], ps[:, 0 : 2 * D])
                nc.vector.scalar_tensor_tensor(
                    out=qsb[:, t, :], in0=ps[:, 2 * D : 3 * D], scalar=1.0,
                    in1=bcat[:, layer, 0:D], op0=OP.mult, op1=OP.add,
                )
                nc.vector.scalar_tensor_tensor(
                    out=xr[:, t, :], in0=ps[:, 3 * D : 4 * D], scalar=1.0,
                    in1=bcat[:, layer, D : 2 * D], op0=OP.mult, op1=OP.add,
                )
                nc.sync.dma_start(kvb.ap()[t * P : (t + 1) * P, :], kvq[:])

            def _final_block(t):
                """Final LN on h[:,t] -> out DMA."""
                ot = sp.tile([P, D], dt.float32, tag="ot")
                stats = sp.tile([P, 6], dt.float32, tag="bst")
                nc.vector.bn_stats(stats[:], h[:, t, :])
                mv = sp.tile([P, 2], dt.float32, tag="mv")
                nc.vector.bn_aggr(mv[:], stats[:])
                ve = sp.tile([P, 1], dt.float32, tag="ve")
                nc.vector.tensor_scalar_add(ve[:], mv[:, 1:2], 1e-5)
                rs = sp.tile([P, 1], dt.float32, tag="rs")
                _rsqrt(rs, ve)
                nc.vector.tensor_tensor(
                    out=ot[:], in0=h[:, t, :],
                    in1=mv[:, 0:1].to_broadcast([P, D]), op=OP.subtract,
                )
                nc.vector.scalar_tensor_tensor(
                    out=ot[:], in0=ot[:], scalar=rs[:], in1=fnp[:, 0:D],
                    op0=OP.mult, op1=OP.mult,
                )
                nc.vector.tensor_tensor(
                    out=ot[:], in0=ot[:], in1=fnp[:, D : 2 * D], op=OP.add
                )
                lo = t * P
                hi = min((t + 1) * P, NSH)
                if hi > lo:
                    nc.sync.dma_start(out_p.ap()[lo:hi, :], ot[0 : hi - lo, :])

            # ---- phase 0: h = x @ W_in + b_in + emb_in[idg] + emb_out[odg]
            for t in range(NBLK):
                xt = sp.tile([P, D], dt.float32, tag="xt")
                nc.sync.dma_start(xt[:], xin.ap()[t * P : (t + 1) * P, :])
                xb = sp.tile([P, D], dt.bfloat16, tag="xb")
                nc.vector.tensor_copy(xb[:], xt[:])
                pT = psA.tile([P, P], dt.bfloat16, tag="pT")
                nc.tensor.transpose(out=pT[:], in_=xb[:], identity=ident[:])
                xTb = sp.tile([P, D], dt.bfloat16, tag="xTb")
                nc.scalar.copy(xTb[:], pT[:])
                ph = psB.tile([P, D], dt.float32, tag="ph")
                nc.tensor.matmul(out=ph[:], lhsT=xTb[:], rhs=win[:], start=True, stop=True)
                nc.vector.scalar_tensor_tensor(
                    out=h[:, t, :], in0=ph[:], scalar=1.0, in1=bin_t[:],
                    op0=OP.mult, op1=OP.add,
                )
            for tabl, idxp in ((emb_i, idg_p), (emb_o, odg_p)):
                gi = kp.tile([P, NPAD // 16], dt.int16, tag="gidx")
                nc.sync.dma_start(gi[:], idxp.ap())
                for i0 in range(0, NBLK, EBATCH):
                    i1 = min(i0 + EBATCH, NBLK)
                    nidx = (i1 - i0) * P
                    eg = sp.tile([P, EBATCH, D], dt.float32, tag="embg")
                    nc.gpsimd.dma_gather(
                        out_ap=eg[:, 0 : i1 - i0, :], in_ap=tabl.ap(),
                        idxs_ap=gi[:, i0 * 8 : i1 * 8],
                        num_idxs=nidx, num_idxs_reg=nidx, elem_size=D,
                        queue_num=_nextq(),
                    )
                    nc.vector.tensor_tensor(
                        out=h[:, i0:i1, :], in0=h[:, i0:i1, :],
                        in1=eg[:, 0 : i1 - i0, :], op=OP.add,
                    )

            # zero the kvg pool buffers once (trimmed gathers leave stale
            # bytes behind; first use must not see NaN bit patterns)
            for _ in range(GPBUFS):
                z = gp.tile([P, trun, 2 * D], dt.bfloat16, tag="kvg")
                nc.vector.memset(z[:], 0.0)

            # ---- pre-pass: LN + projections for layer 0
            for t in range(NBLK):
                _lnproj_block(t, 0)

            # ---- layers ----------------------------------------------------
            for layer in range(L):
                if PROBE_NO_COLLECTIVE:
                    nc.gpsimd.dma_start(out=kvf.ap()[0:NPAD, :], in_=kvb.ap())
                else:
                    nc.gpsimd.collective_compute(
                        "AllGather",
                        OP.bypass,
                        replica_groups=[list(range(NCORES))],
                        ins=[kvb.ap().opt()],
                        outs=[kvf.ap().opt()],
                    )

                # ---- edge phase: per (tgt block, bucket) run of trun tiles
                for blk in range(NBLK):
                    pm = psC.tile([P, D + H], dt.float32, tag="pm")
                    for b in range(2):
                        off = (blk * 2 + b) * EDB
                        ed = gp.tile([P, EDB], dt.uint8, tag="ed")
                        nc.sync.dma_start(ed[:], ed_p.ap()[:, off : off + EDB])
                        ki = ed[:, 0:KIB].bitcast(dt.int16)         # [P, trun*8]
                        sgv = ed[:, KIB : KIB + nrun].bitcast(dt.float8e4).rearrange(
                            "p (t e) -> p t e", e=P
                        )
                        stv = ed[:, KIB + nrun : EDB].bitcast(dt.float8e4).rearrange(
                            "p (t e) -> p t e", e=P
                        )

                        if ABL_NOEDGE:
                            ue0 = ep.tile([P, trun, D + H], dt.bfloat16, tag="ue")
                            nc.vector.memset(ue0[:], 0.5)
                            for tt in range(trun):
                                nc.tensor.matmul(
                                    out=pm[:], lhsT=sgv[:, tt, :], rhs=ue0[:, tt, :],
                                    start=(b == 0 and tt == 0),
                                    stop=(b == 1 and tt == trun - 1),
                                )
                            continue
                        kvg = gp.tile([P, trun, 2 * D], dt.bfloat16, tag="kvg")
                        for ci, i0 in enumerate(range(0, trun, GBATCH)):
                            i1 = min(i0 + GBATCH, trun)
                            nidx = (i1 - i0) * P
                            gj = (blk * 2 + b) * NC_G + ci
                            greg = gregs[gregc[0] % len(gregs)]
                            gregc[0] += 1
                            nc.gpsimd.reg_load(greg, gct[0:1, gj : gj + 1])
                            nc.gpsimd.dma_gather(
                                out_ap=kvg[:, i0:i1, :],
                                in_ap=kvf.ap()[b * NB_ROWS : (b + 1) * NB_ROWS, :],
                                idxs_ap=ki[:, i0 * 8 : i1 * 8],
                                num_idxs=nidx, num_idxs_reg=greg,
                                elem_size=2 * D,
                                queue_num=_nextq(),
                            )
                        # q-broadcast via PE: qg[e, f] = q[tl(e), f]
                        qg = ep.tile([P, trun, D], dt.bfloat16, tag="qg")
                        for c0 in range(0, nrun, 512):
                            c1 = min(c0 + 512, nrun)
                            qp = psQ.tile([P, 512], dt.float32, tag="qp")
                            for tt in range(c0 // P, c1 // P):
                                o = tt * P - c0
                                nc.tensor.matmul(
                                    out=qp[:, o : o + P], lhsT=stv[:, tt, :],
                                    rhs=qsb[:, blk, :], start=True, stop=True,
                                )
                            nc.scalar.copy(
                                qg[:, c0 // P : c1 // P, :],
                                qp[:, 0 : c1 - c0].rearrange("p (t e) -> p t e", e=P),
                            )
                        # per-edge logits: alpha = sum_c q*k (tree reduce)
                        qk = ep.tile([P, trun, H, C], dt.bfloat16, tag="qk")
                        nc.vector.tensor_tensor(
                            out=qk[:].rearrange("p t h c -> p t (h c)"),
                            in0=qg[:], in1=kvg[:, :, 0:D], op=OP.mult,
                        )
                        t1 = ep.tile([P, trun, H, 4], dt.bfloat16, tag="t1")
                        with nc.allow_low_precision(reason="alpha logits are O(0.1)"):
                            nc.vector.tensor_tensor(
                                out=t1[:], in0=qk[:, :, :, 0:4], in1=qk[:, :, :, 4:8],
                                op=OP.add,
                            )
                            t2 = ep.tile([P, trun, H, 2], dt.bfloat16, tag="t2")
                            nc.vector.tensor_tensor(
                                out=t2[:], in0=t1[:, :, :, 0:2], in1=t1[:, :, :, 2:4],
                                op=OP.add,
                            )
                            al = ep.tile([P, trun, H, 1], dt.bfloat16, tag="al")
                            nc.vector.tensor_tensor(
                                out=al[:], in0=t2[:, :, :, 0:1], in1=t2[:, :, :, 1:2],
                                op=OP.add,
                            )
                        ue = ep.tile([P, trun, D + H], dt.bfloat16, tag="ue")
                        nc.scalar.activation(
                            out=ue[:, :, D : D + H].rearrange("p t (h o) -> p t h o", o=1),
                            in_=al[:], func=AF.Exp,
                        )
                        wex = ep.tile([P, trun, H, C], dt.bfloat16, tag="wex")
                        nc.scalar.activation(
                            out=wex[:], in_=al[:].to_broadcast([P, trun, H, C]),
                            func=AF.Exp,
                        )
                        nc.vector.tensor_tensor(
                            out=ue[:, :, 0:D], in0=kvg[:, :, D : 2 * D],
                            in1=wex[:].rearrange("p t h c -> p t (h c)"), op=OP.mult,
                        )
                        for tt in range(trun):
                            nc.tensor.matmul(
                                out=pm[:], lhsT=sgv[:, tt, :], rhs=ue[:, tt, :],
                                start=(b == 0 and tt == 0),
                                stop=(b == 1 and tt == trun - 1),
                            )

                    # ---- fused per-block tail: normalize, gate, residual,
                    # then next layer's LN+projection (or final LN) ----------
                    msgb = sp.tile([P, D], dt.float32, tag="msgb")
                    rden = sp.tile([P, H, 1], dt.float32, tag="rden")
                    nc.vector.tensor_scalar_add(
                        rden[:], pm[:, D : D + H].rearrange("p (h o) -> p h o", o=1), 1e-20
                    )
                    nc.vector.reciprocal(rden[:], rden[:])
                    nc.vector.tensor_tensor(
                        out=msgb[:].rearrange("p (h c) -> p h c", c=C),
                        in0=pm[:, 0:D].rearrange("p (h c) -> p h c", c=C),
                        in1=rden[:].to_broadcast([P, H, C]),
                        op=OP.mult,
                    )
                    nc.vector.tensor_tensor(
                        out=msgb[:], in0=msgb[:], in1=bvt[:, layer, :], op=OP.add
                    )
                    scr = sp.tile([P, D], dt.float32, tag="scr")
                    bs1 = sp.tile([P, 1], dt.float32, tag="bs1")
                    nc.vector.scalar_tensor_tensor(
                        out=scr[:], in0=msgb[:], scalar=1.0, in1=wb[:, layer, 0:D],
                        op0=OP.mult, op1=OP.mult, accum_out=bs1[:],
                    )
                    bs2 = sp.tile([P, 1], dt.float32, tag="bs2")
                    nc.vector.scalar_tensor_tensor(
                        out=scr[:], in0=xr[:, blk, :], scalar=1.0, in1=wb[:, layer, D : 2 * D],
                        op0=OP.mult, op1=OP.mult, accum_out=bs2[:],
                    )
                    nc.vector.tensor_tensor(out=bs1[:], in0=bs1[:], in1=bs2[:], op=OP.add)
                    beta = sp.tile([P, 1], dt.float32, tag="beta")
                    nc.scalar.activation(out=beta[:], in_=bs1[:], func=AF.Exp, scale=-1.0)
                    nc.vector.tensor_scalar_add(beta[:], beta[:], 1.0)
                    nc.vector.reciprocal(beta[:], beta[:])
                    # h += msg + beta*(xr - msg)
                    tmpb = sp.tile([P, D], dt.float32, tag="tmpb")
                    nc.vector.tensor_tensor(
                        out=tmpb[:], in0=xr[:, blk, :], in1=msgb[:], op=OP.subtract
                    )
                    nc.vector.scalar_tensor_tensor(
                        out=tmpb[:], in0=tmpb[:], scalar=beta[:], in1=msgb[:],
                        op0=OP.mult, op1=OP.add,
                    )
                    nc.vector.tensor_tensor(
                        out=h[:, blk, :], in0=h[:, blk, :], in1=tmpb[:], op=OP.add
                    )
                    if layer == L - 1:
                        _final_block(blk)
                    else:
                        _lnproj_block(blk, layer + 1)

    nc.finalize()
    return nc

LAST_RES = None


def _make_in_maps(inputs, cores):
    sq = 1.0 / np.sqrt(np.float32(C))
    Wq, Wk, Wv, Wsk = (np.asarray(inputs[k], dtype=np.float32) for k in ("Wq", "Wk", "Wv", "Wskip"))
    bq, bv, bsk = (np.asarray(inputs[k], dtype=np.float32) for k in ("bq", "bv", "bskip"))
    # order per layer: k | v | q*sq | skip  (k-bias dropped: softmax shift
    # invariance; v-bias folded in post-aggregation)
    wcat = np.concatenate([Wk, Wv, Wq * sq, Wsk], axis=2).transpose(1, 0, 2).reshape(D, L * 4 * D)
    bcat = np.concatenate([bq * sq, bsk], axis=1)  # [L, 2D]
    bcat_rep = np.broadcast_to(bcat[:, None, :], (L, P, 2 * D)).transpose(1, 0, 2).reshape(P, L * 2 * D).copy()
    bvp = np.broadcast_to(bv[:, None, :], (L, P, D)).transpose(1, 0, 2).reshape(P, L * D).copy()
    lns, lnb = np.asarray(inputs["ln_scale"], np.float32), np.asarray(inputs["ln_bias"], np.float32)
    lnp = np.broadcast_to(
        np.concatenate([lns, lnb], axis=1)[:, None, :], (L, P, 2 * D)
    ).transpose(1, 0, 2).reshape(P, L * 2 * D).copy()
    fnp = np.broadcast_to(
        np.concatenate([inputs["fn_scale"], inputs["fn_bias"]])[None, :], (P, 2 * D)
    ).astype(np.float32).copy()
    Wbeta = np.asarray(inputs["Wbeta"], np.float32)  # [L, 3D, 1]
    wa = Wbeta[:, 0:D, 0] + Wbeta[:, 2 * D : 3 * D, 0]      # msg coeff
    wbx = Wbeta[:, D : 2 * D, 0] - Wbeta[:, 2 * D : 3 * D, 0]  # xr coeff
    wbeta_rep = np.broadcast_to(
        np.concatenate([wa, wbx], axis=1)[:, None, :], (L, P, 2 * D)
    ).transpose(1, 0, 2).reshape(P, L * 2 * D).copy()
    bin_rep = np.broadcast_to(
        np.asarray(inputs["b_in"], np.float32)[None, :], (P, D)
    ).copy()

    common = dict(
        emb_in=np.asarray(inputs["in_emb"], np.float32),
        emb_out=np.asarray(inputs["out_emb"], np.float32),
        win=_bf16(inputs["W_in"]),
        bin=bin_rep,
        wcat=_bf16(wcat),
        bcat=bcat_rep,
        bvp=bvp,
        lnp=lnp,
        fnp=fnp,
        wbeta=wbeta_rep,
    )
    in_maps = []
    for c in range(NCORES):
        m = dict(common)
        cd = cores[c]
        m.update(x=cd["x"], idg=cd["idg"], odg=cd["odg"], edata=cd["edata"],
                 gcnt=cd["gcnt"])
        in_maps.append(m)
    return in_maps


def kernel(**inputs):
    import os

    from concourse.bass_utils import run_bass_kernel_spmd

    x = np.asarray(inputs["x"], dtype=np.float32)
    edge_index = np.asarray(inputs["edge_index"])
    cores, trun, EDB = _preprocess(x, edge_index)
    in_maps = _make_in_maps(inputs, cores)

    nc = _build(trun, EDB)
    kw = {}
    td = os.environ.get("BASS_KERNEL_TMPDIR")
    if td:
        kw["tmpdir"] = td
    res = run_bass_kernel_spmd(nc, in_maps, core_ids=list(range(NCORES)), **kw)
    global LAST_RES
    LAST_RES = res
    outs = [np.asarray(r["out"], dtype=np.float32) for r in res.results]
    return np.concatenate(outs, axis=0)


if __name__ == "__main__":
    import reference

    inp = {k: np.asarray(v) for k, v in reference.setup_inputs().items()}
    exp = np.asarray(reference.reference(**inp))
    act = kernel(**inp)
    err = np.abs(act - exp).max() / (np.abs(exp).max() + 1e-9)
    print("Relative error:", err)
